# revision 1
# baseline (speedup 1.0000x reference)
# Trainium2 Bass kernels for nn_DecoderLayer (T5-style decoder layer).
# Two SPMD launches over 8 cores:
#   A: head-parallel attention (2 heads/core) -> per-head attn outputs O^T
#   B: token-parallel output-proj + FFN (512 tokens/core)
# Activations kept feature-major (features on partitions).
import sys
sys.path.insert(0, '/opt/trn_rl_repo')
import math
import numpy as np
import bass_rust
import concourse.bass as bass
import concourse.mybir as mybir
import concourse.tile as tile
from concourse import bacc

F32 = mybir.dt.float32
F32R = mybir.dt.float32r
AF = mybir.ActivationFunctionType

E = 1024
H = 16
D = 64
HID = 4096
B = 2
S = 2048
NB = 32
MAXD = 128
LN_EPS = 1e-6
KQ_EPS = 1e-6
NCORE = 8
HPC = H // NCORE          # heads per core (2)
TPC = B * S // NCORE      # tokens per core (512)
NQC = S // 512            # q-chunks per batch (4)
BOFF = 1024               # bias vec offset: index j = d + BOFF
MASKVAL = -20.0


def rel_bucket(d):
    d = np.asarray(d)
    max_exact = NB // 2
    safe = np.maximum(d, 1).astype(np.float64)
    large = max_exact + (
        np.log(safe / max_exact) / math.log(MAXD / max_exact) * (NB - max_exact)
    ).astype(np.int32)
    large = np.minimum(large, NB - 1)
    return np.where(d < max_exact, d, large)


def build_launch_a():
    nc = bacc.Bacc("TRN2", target_bir_lowering=False, debug=False)
    xT_d = nc.dram_tensor("xT", [E, B * S], F32R, kind="ExternalInput").ap()
    wq_d = nc.dram_tensor("wq", [128, 8 * HPC * D], F32R, kind="ExternalInput").ap()
    wk_d = nc.dram_tensor("wk", [128, 8 * HPC * D], F32R, kind="ExternalInput").ap()
    wv_d = nc.dram_tensor("wv", [128, 8 * HPC * D], F32R, kind="ExternalInput").ap()
    biast_d = nc.dram_tensor("biast", [HPC * 5, 128, 512], F32,
                             kind="ExternalInput")
    fconst_d = nc.dram_tensor("fconst", [128, HPC], F32, kind="ExternalInput").ap()
    scale2_d = nc.dram_tensor("scale2", [HPC, 1], F32, kind="ExternalInput").ap()
    hsel_d = nc.dram_tensor("hsel", [HPC, 128], F32R, kind="ExternalInput").ap()
    onesc_d = nc.dram_tensor("onesc", [128, 1], F32R, kind="ExternalInput").ap()
    onesr_d = nc.dram_tensor("onesr", [1, 128], F32R, kind="ExternalInput").ap()
    ones64_d = nc.dram_tensor("ones64", [1, 64], F32R, kind="ExternalInput").ap()
    hsum_d = nc.dram_tensor("hsum", [128, HPC], F32R, kind="ExternalInput").ap()
    identc_d = nc.dram_tensor("identc", [128, 64], F32R, kind="ExternalInput").ap()
    oT_d = nc.dram_tensor("oT", [HPC * D, B * S], F32, kind="ExternalOutput").ap()

    with tile.TileContext(nc) as tc:
        with nc.allow_low_precision(reason="fp32r kernel"), \
             tc.tile_pool(name="const", bufs=1) as cpool, \
             tc.tile_pool(name="xt", bufs=1) as xpool, \
             tc.tile_pool(name="w", bufs=1) as wpool, \
             tc.tile_pool(name="qkv", bufs=1) as qkvpool, \
             tc.tile_pool(name="rawp", bufs=2) as rawpool, \
             tc.tile_pool(name="vtok", bufs=1) as vpool, \
             tc.tile_pool(name="ptile", bufs=4) as ppool, \
             tc.tile_pool(name="onorm", bufs=3) as opool, \
             tc.tile_pool(name="small", bufs=2) as spool, \
             tc.tile_pool(name="r1p", bufs=1) as rpool, \
             tc.tile_pool(name="ps_mm", bufs=4, space="PSUM") as ps_mm, \
             tc.tile_pool(name="ps_acc", bufs=2, space="PSUM") as ps_acc:

            # ---- constants ----
            fconst_t = cpool.tile([128, HPC], F32)
            nc.sync.dma_start(fconst_t[:], fconst_d[:])
            scale2_t = cpool.tile([HPC, 1], F32)
            nc.sync.dma_start(scale2_t[:], scale2_d[:])
            ones128 = cpool.tile([128, 1], F32R)
            nc.sync.dma_start(ones128[:], onesc_d[:])
            ones1x64 = cpool.tile([1, 64], F32R)
            nc.sync.dma_start(ones1x64[:], ones64_d[:])
            onesr = cpool.tile([1, 128], F32R)
            nc.sync.dma_start(onesr[:], onesr_d[:])
            hsum = cpool.tile([128, HPC], F32R)
            nc.sync.dma_start(hsum[:], hsum_d[:])
            # head-select [2, 128]: row h = 1 on cols h*64..
            hsel = cpool.tile([HPC, 128], F32R)
            nc.sync.dma_start(hsel[:], hsel_d[:])
            # stacked identities [128, 64] (f32r) for per-head PE transpose
            epsln_t = cpool.tile([128, 1], F32)
            nc.vector.memset(epsln_t[:], LN_EPS)
            epskq_t = cpool.tile([128, 1], F32)
            nc.vector.memset(epskq_t[:], KQ_EPS)
            ident = cpool.tile([128, 64], F32R)
            nc.sync.dma_start(ident[:], identc_d[:])
            # near-band bias tiles: [128, (h*5+di)*512 + f]
            biast = cpool.tile([128, HPC * 5 * 512], F32)
            nc.sync.dma_start(
                biast[:],
                bass_rust.AP(biast_d, 0, [[512, 128], [65536, HPC * 5],
                                          [1, 512]]))

            # weights (ln1-folded, f32r): [128, e-tile, col]
            wq_t = wpool.tile([128, 8, HPC * D], F32R, tag="wq")
            wk_t = wpool.tile([128, 8, HPC * D], F32R, tag="wk")
            wv_t = wpool.tile([128, 8, HPC * D], F32R, tag="wv")
            for (w_t, w_d) in ((wq_t, wq_d), (wk_t, wk_d), (wv_t, wv_d)):
                nc.sync.dma_start(
                    w_t[:], w_d.rearrange("p (i m) -> p i m", i=8))

            for b in range(B):
                # ---- xT e-tiles [128, 2048] ----
                xts = []
                for i in range(8):
                    xt = xpool.tile([128, S], F32R, tag=f"x{i}")
                    nc.sync.dma_start(xt[:], xT_d[i * 128:(i + 1) * 128,
                                                  b * S:(b + 1) * S])
                    xts.append(xt)

                # ---- r1 = 1/sqrt(mean(x^2)+eps) ----
                r1 = rpool.tile([1, S], F32R, tag="r1")
                for ch in range(S // 512):
                    csl = slice(ch * 512, (ch + 1) * 512)
                    ssx_ps = ps_acc.tile([128, 512], F32, tag="acc")
                    for i in range(8):
                        sq = spool.tile([128, 512], F32R, tag="sqx")
                        nc.scalar.activation(sq[:], xts[i][:, csl], AF.Square)
                        nc.tensor.matmul(ssx_ps[0:1, :], ones128[:], sq[:],
                                         start=(i == 0), stop=(i == 7))
                    r1sq = spool.tile([1, 512], F32, tag="r1sq")
                    nc.scalar.activation(r1sq[:], ssx_ps[0:1, :], AF.Sqrt,
                                         bias=epsln_t[0:1, :], scale=1.0 / E)
                    nc.vector.reciprocal(r1[:, csl], r1sq[:])

                # ---- projections: raw qT/kT/vT packed [128, 2048] ----
                def project(w_t, name):
                    pool_ = rawpool if name == "raw" else qkvpool
                    out = pool_.tile([128, S], F32R, tag=name)
                    for ch in range(S // 512):
                        csl = slice(ch * 512, (ch + 1) * 512)
                        ps = ps_mm.tile([128, 512], F32, tag="mm")
                        for i in range(8):
                            nc.tensor.matmul(
                                ps[:], w_t[:, i, :], xts[i][:, csl],
                                start=(i == 0), stop=(i == 7))
                        nc.scalar.activation(out[:, csl], ps[:], AF.Copy)
                    return out

                qT_raw = project(wq_t, "raw")
                kT_raw = project(wk_t, "raw")
                vT_raw = project(wv_t, "vT_raw")

                # ---- L2-normalize q (scale-folded) and k ----
                def l2norm(raw, name, fold_scale):
                    out = qkvpool.tile([128, S], F32R, tag=name)
                    for ch in range(S // 512):
                        csl = slice(ch * 512, (ch + 1) * 512)
                        sq = spool.tile([128, 512], F32R, tag="sqn")
                        nc.scalar.activation(sq[:], raw[:, csl], AF.Square)
                        ss_ps = ps_mm.tile([128, 512], F32, tag="mm")
                        nc.tensor.matmul(ss_ps[0:HPC, :], hsum[:], sq[:],
                                         start=True, stop=True)
                        rnsq = spool.tile([HPC, 512], F32, tag="rnsq")
                        nc.scalar.activation(rnsq[:], ss_ps[0:HPC, :], AF.Sqrt,
                                             bias=epskq_t[0:HPC, :])
                        rn = spool.tile([HPC, 512], F32R, tag="rn")
                        nc.vector.reciprocal(rn[:], rnsq[:])
                        if fold_scale:
                            nc.vector.tensor_scalar_mul(rn[:], rn[:],
                                                        scale2_t[:])
                        bc_ps = ps_mm.tile([128, 512], F32, tag="mm")
                        nc.tensor.matmul(bc_ps[:], hsel[:], rn[:],
                                         start=True, stop=True)
                        nc.vector.tensor_mul(out[:, csl], raw[:, csl], bc_ps[:])
                    return out

                qT = l2norm(qT_raw, "qT", True)
                kT = l2norm(kT_raw, "kT", False)

                # ---- v scaled by r1 in place (feature-major) ----
                vT_s = vT_raw
                mvv = spool.tile([128, 1], F32, tag="mvv")
                mv4 = spool.tile([128, NQC], F32, tag="mv4")
                for ch in range(NQC):
                    csl = slice(ch * 512, (ch + 1) * 512)
                    bc_ps = ps_mm.tile([128, 512], F32, tag="mm")
                    nc.tensor.matmul(bc_ps[:], onesr[:], r1[:, csl],
                                     start=True, stop=True)
                    nc.vector.tensor_mul(vT_s[:, csl], vT_raw[:, csl], bc_ps[:])
                    nc.vector.reduce_sum(mv4[:, ch:ch + 1], vT_s[:, csl],
                                         mybir.AxisListType.X)
                nc.vector.reduce_sum(mvv[:], mv4[:], mybir.AxisListType.X)

                v_augs = [[], []]
                for kt in range(S // 128):
                    for h in range(HPC):
                        va = vpool.tile([128, D + 1], F32R, tag=f"va{h}_{kt}")
                        nc.vector.tensor_copy(va[:, D:D + 1], ones128[:])
                        tr_ps = ps_mm.tile([128, 512], F32R, tag="mm")
                        nc.tensor.transpose(
                            tr_ps[0:128, 0:64],
                            vT_s[h * D:(h + 1) * D, kt * 128:(kt + 1) * 128],
                            ident[h * D:(h + 1) * D, :])
                        nc.vector.tensor_copy(va[:, 0:D], tr_ps[0:128, 0:64])
                        v_augs[h].append(va)

                # ---- attention ----
                for h in range(HPC):
                    hd = slice(h * D, (h + 1) * D)
                    for qc in range(NQC):
                        Q0 = qc * 512
                        qsl = slice(Q0, Q0 + 512)
                        nkt = (Q0 + 512) // 128
                        o_ps = ps_acc.tile([128, 512], F32, tag="acc")
                        for kt in range(nkt):
                            K0 = kt * 128
                            s_ps = ps_mm.tile([128, 512], F32, tag="mm")
                            nc.tensor.matmul(
                                s_ps[:], kT[hd, K0:K0 + 128], qT[hd, qsl],
                                start=True, stop=True)
                            p_t = ppool.tile([128, 512], F32R, tag="p")
                            if Q0 - K0 >= 255:
                                nc.scalar.activation(
                                    p_t[:], s_ps[:], AF.Exp,
                                    bias=fconst_t[:, h:h + 1])
                            else:
                                di = 1 + kt - 4 * qc
                                bsl = slice((h * 5 + di) * 512,
                                            (h * 5 + di + 1) * 512)
                                nc.vector.tensor_add(s_ps[:], s_ps[:],
                                                     biast[:, bsl])
                                nc.scalar.activation(p_t[:], s_ps[:], AF.Exp)
                            nc.tensor.matmul(
                                o_ps[0:D + 1, :], v_augs[h][kt][:], p_t[:],
                                start=(kt == 0), stop=(kt == nkt - 1))
                        srec = spool.tile([1, 512], F32R, tag="srec")
                        nc.vector.reciprocal(srec[:], o_ps[D:D + 1, :])
                        nb_ps = ps_mm.tile([128, 512], F32, tag="mm")
                        nc.tensor.matmul(nb_ps[0:D, :], ones1x64[:], srec[:],
                                         start=True, stop=True)
                        nb = spool.tile([64, 512], F32, tag="nbs")
                        nc.scalar.activation(nb[:], nb_ps[0:D, :], AF.Copy)
                        o_n = opool.tile([64, 512], F32, tag="on")
                        nc.vector.tensor_mul(o_n[:], o_ps[0:D, :], nb[:])
                        if qc == 0:
                            nc.scalar.activation(
                                o_n[:, 0:1], mvv[h * D:(h + 1) * D, :],
                                AF.Copy, scale=1.0 / S)
                        nc.sync.dma_start(
                            oT_d[hd, b * S + Q0: b * S + Q0 + 512], o_n[:])
    nc.compile()
    return nc


def prep_a_inputs(inputs):
    x = np.asarray(inputs["x"], np.float32)
    ln1 = np.asarray(inputs["ln1_w"], np.float32)
    Wq = np.asarray(inputs["Wq"], np.float32)
    Wk = np.asarray(inputs["Wk"], np.float32)
    Wv = np.asarray(inputs["Wv"], np.float32)
    rb = np.asarray(inputs["rel_bias"], np.float32)
    scale = np.asarray(inputs["scale"], np.float32)
    xT = np.ascontiguousarray(x.reshape(B * S, E).T)
    d = np.arange(2048) - BOFF
    bucket = rel_bucket(np.maximum(d, 1))
    biasv_all = np.where(
        (d < 1)[None, :], np.float32(MASKVAL),
        scale[:, None] * rb[bucket, :].T.astype(np.float32)).astype(np.float32)
    # bias delta-tiles: [H, 5, 128, 512]; B[h,di,p,f] = v_h[BOFF+128-128*di+f-p]
    di_ = np.arange(5)[:, None, None]
    p_ = np.arange(128)[None, :, None]
    f_ = np.arange(512)[None, None, :]
    idx = BOFF + 128 - 128 * di_ + f_ - p_
    biast_all = biasv_all[:, idx]  # [H, 5, 128, 512]
    fconst_all = (scale * rb[NB - 1, :]).astype(np.float32)

    def tile_w(w):  # [1024, M] -> [128, 8*M]
        M = w.shape[1]
        return np.ascontiguousarray(
            w.reshape(8, 128, M).transpose(1, 0, 2).reshape(128, 8 * M))
    in_maps = []
    for c in range(NCORE):
        hs = slice(c * HPC, (c + 1) * HPC)
        cs = slice(c * HPC * D, (c + 1) * HPC * D)
        hsel_np = np.zeros((HPC, 128), np.float32)
        for h in range(HPC):
            hsel_np[h, h * D:(h + 1) * D] = 1.0
        ident_np = np.concatenate([np.eye(D, dtype=np.float32)] * 2, axis=0)
        hsum_np = np.zeros((128, HPC), np.float32)
        for h in range(HPC):
            hsum_np[h * D:(h + 1) * D, h] = 1.0
        in_maps.append({
            "xT": xT,
            "hsel": hsel_np,
            "identc": ident_np,
            "onesc": np.ones((128, 1), np.float32),
            "onesr": np.ones((1, 128), np.float32),
            "ones64": np.ones((1, 64), np.float32),
            "hsum": hsum_np,
            "wq": tile_w(ln1[:, None] * Wq[:, cs]),
            "wk": tile_w(ln1[:, None] * Wk[:, cs]),
            "wv": tile_w(ln1[:, None] * Wv[:, cs]),
            "biast": np.ascontiguousarray(
                biast_all[hs].reshape(HPC * 5, 128, 512)),
            "fconst": np.ascontiguousarray(
                np.broadcast_to(fconst_all[hs], (128, HPC))),
            "scale2": np.ascontiguousarray(scale[hs, None]),
        })
    return in_maps


def build_launch_b():
    nc = bacc.Bacc("TRN2", target_bir_lowering=False, debug=False)
    oT_d = nc.dram_tensor("oTs", [E, TPC], F32R, kind="ExternalInput").ap()
    xT_d = nc.dram_tensor("xTs", [E, TPC], F32R, kind="ExternalInput").ap()
    wo_d = nc.dram_tensor("wo", [E // 128, 128, E], F32R, kind="ExternalInput")
    w1_d = nc.dram_tensor("w1", [HID // 128, 128, E], F32R, kind="ExternalInput")
    w2_d = nc.dram_tensor("w2", [E // 128, 128, HID], F32R, kind="ExternalInput")
    onesc_d = nc.dram_tensor("onesc", [128, 1], F32R, kind="ExternalInput").ap()
    onesr_d = nc.dram_tensor("onesr", [1, 128], F32R, kind="ExternalInput").ap()
    out_d = nc.dram_tensor("outT", [E, TPC], F32, kind="ExternalOutput").ap()

    NE = E // 128    # 8 e-tiles
    NH = HID // 128  # 32 h-tiles

    with tile.TileContext(nc) as tc:
        with nc.allow_low_precision(reason="fp32r kernel"), \
             tc.tile_pool(name="const", bufs=1) as cpool, \
             tc.tile_pool(name="io", bufs=1) as iopool, \
             tc.tile_pool(name="y", bufs=1) as ypool, \
             tc.tile_pool(name="h", bufs=1) as hpool, \
             tc.tile_pool(name="w1s", bufs=2) as w1pool, \
             tc.tile_pool(name="w2s", bufs=2) as w2pool, \
             tc.tile_pool(name="small", bufs=2) as spool, \
             tc.tile_pool(name="r1p", bufs=1) as rpool, \
             tc.tile_pool(name="ps_mm", bufs=4, space="PSUM") as ps_mm, \
             tc.tile_pool(name="ps_acc", bufs=2, space="PSUM") as ps_acc:

            ones128 = cpool.tile([128, 1], F32R)
            nc.sync.dma_start(ones128[:], onesc_d[:])
            epsln_t = cpool.tile([128, 1], F32)
            nc.vector.memset(epsln_t[:], LN_EPS)

            oTs, xTs = [], []
            for i in range(NE):
                ot = iopool.tile([128, TPC], F32R, tag=f"o{i}")
                nc.sync.dma_start(ot[:], oT_d[i * 128:(i + 1) * 128, :])
                oTs.append(ot)
                xt = iopool.tile([128, TPC], F32R, tag=f"xs{i}")
                nc.sync.dma_start(xt[:], xT_d[i * 128:(i + 1) * 128, :])
                xTs.append(xt)
            # ---- y^T = Wo^T @ O^T + x^T ----
            yTs = []
            for i in range(NE):
                wo_t = w1pool.tile([128, NE, 128], F32R, tag="w1")
                nc.sync.dma_start(
                    wo_t[:], bass_rust.AP(wo_d, i * 128 * E,
                                          [[E, 128], [1, NE * 128]]))
                ps = ps_acc.tile([128, TPC], F32, tag="acc")
                for j in range(NE):
                    nc.tensor.matmul(ps[:], wo_t[:, j, :],
                                     oTs[j][:], start=(j == 0), stop=(j == NE - 1))
                yt = ypool.tile([128, TPC], F32R, tag=f"y{i}")
                nc.vector.tensor_add(yt[:], ps[:], xTs[i][:])
                yTs.append(yt)

            # ---- r2 = 1/sqrt(mean(y^2)+eps) ----
            ssy_ps = ps_acc.tile([128, TPC], F32, tag="acc")
            for i in range(NE):
                sq = spool.tile([128, TPC], F32R, tag="sqy")
                nc.scalar.activation(sq[:], yTs[i][:], AF.Square)
                nc.tensor.matmul(ssy_ps[0:1, :], ones128[:], sq[:],
                                 start=(i == 0), stop=(i == NE - 1))
            r2sq = spool.tile([1, TPC], F32, tag="r2sq")
            nc.scalar.activation(r2sq[:], ssy_ps[0:1, :], AF.Sqrt,
                                 bias=epsln_t[0:1, :], scale=1.0 / E)
            r2 = spool.tile([1, TPC], F32R, tag="r2")
            nc.vector.reciprocal(r2[:], r2sq[:])
            # broadcast r2 over 128 partitions
            r2b_ps = ps_mm.tile([128, TPC], F32, tag="mm")
            o1x128 = cpool.tile([1, 128], F32R)
            nc.sync.dma_start(o1x128[:], onesr_d[:])
            nc.tensor.matmul(r2b_ps[:], o1x128[:], r2[:], start=True, stop=True)
            r2b = cpool.tile([128, TPC], F32)
            nc.scalar.activation(r2b[:], r2b_ps[:], AF.Copy)

            # ---- h^T = relu(W1'^T y^T) ----
            hts = []
            for ht in range(NH):
                w1_t = w1pool.tile([128, NE, 128], F32R, tag="w1")
                nc.sync.dma_start(
                    w1_t[:], bass_rust.AP(w1_d, ht * 128 * E,
                                          [[E, 128], [1, NE * 128]]))
                ps = ps_mm.tile([128, TPC], F32, tag="mm")
                for j in range(NE):
                    nc.tensor.matmul(ps[:], w1_t[:, j, :], yTs[j][:],
                                     start=(j == 0), stop=(j == NE - 1))
                h_t = hpool.tile([128, TPC], F32R, tag=f"h{ht}")
                nc.scalar.activation(h_t[:], ps[:], AF.Relu)
                hts.append(h_t)

            # ---- z = (h^T' W2)^T * r2 + y ----
            for i in range(NE):
                w2_t = w2pool.tile([128, NH, 128], F32R, tag="w2")
                nc.sync.dma_start(
                    w2_t[:], bass_rust.AP(w2_d, i * 128 * HID,
                                          [[HID, 128], [1, NH * 128]]))
                ps = ps_acc.tile([128, TPC], F32, tag="acc")
                for ht in range(NH):
                    nc.tensor.matmul(ps[:], w2_t[:, ht, :], hts[ht][:],
                                     start=(ht == 0), stop=(ht == NH - 1))
                zt = spool.tile([128, TPC], F32, tag="zt")
                nc.vector.tensor_mul(zt[:], ps[:], r2b[:])
                outt = spool.tile([128, TPC], F32, tag="outt")
                nc.vector.tensor_add(outt[:], zt[:], yTs[i][:])
                nc.sync.dma_start(out_d[i * 128:(i + 1) * 128, :], outt[:])
    nc.compile()
    return nc


def prep_b_inputs(inputs, oT_all):
    x = np.asarray(inputs["x"], np.float32)
    ln2 = np.asarray(inputs["ln2_w"], np.float32)
    def tile_cols(w):
        # [K, M] -> [M//128, 128, K]: out[i, p, j*128+m] = w[j*128+p, i*128+m]
        K, M = w.shape
        return np.ascontiguousarray(
            w.reshape(K // 128, 128, M // 128, 128)
            .transpose(2, 1, 0, 3).reshape(M // 128, 128, K))
    Wo = tile_cols(np.asarray(inputs["Wo"], np.float32))
    W1 = tile_cols(ln2[:, None] * np.asarray(inputs["W1"], np.float32))
    W2 = tile_cols(np.asarray(inputs["W2"], np.float32))
    xT = x.reshape(B * S, E).T
    in_maps = []
    for c in range(NCORE):
        ts = slice(c * TPC, (c + 1) * TPC)
        in_maps.append({
            "oTs": np.ascontiguousarray(oT_all[:, ts]),
            "onesc": np.ones((128, 1), np.float32),
            "onesr": np.ones((1, 128), np.float32),
            "xTs": np.ascontiguousarray(xT[:, ts]),
            "wo": Wo, "w1": W1, "w2": W2,
        })
    return in_maps


_CACHE = {}


def _get_compiled():
    if "a" not in _CACHE:
        _CACHE["a"] = build_launch_a()
    if "b" not in _CACHE:
        _CACHE["b"] = build_launch_b()
    return _CACHE["a"], _CACHE["b"]


def kernel(**inputs):
    from concourse import bass_utils
    inputs = {k: np.asarray(v) for k, v in inputs.items()}
    nca, ncb = _get_compiled()
    in_maps_a = prep_a_inputs(inputs)
    res_a = bass_utils.run_bass_kernel_spmd(
        nca, in_maps_a, core_ids=list(range(NCORE)))
    oT_all = np.concatenate([res_a.results[c]["oT"] for c in range(NCORE)],
                            axis=0)  # [E, B*S], rows = h*64+d
    in_maps_b = prep_b_inputs(inputs, oT_all)
    res_b = bass_utils.run_bass_kernel_spmd(
        ncb, in_maps_b, core_ids=list(range(NCORE)))
    outT = np.concatenate([res_b.results[c]["outT"] for c in range(NCORE)],
                          axis=1)  # [E, B*S]
    return np.ascontiguousarray(outT.T).reshape(B, S, E).astype(np.float32)



# revision 6
# speedup vs baseline: 1.2098x; 1.2098x over previous
# Trainium2 Bass kernels for nn_DecoderLayer (T5-style decoder layer).
# Two SPMD launches over 8 cores:
#   A: head-parallel attention (2 heads/core) -> per-head attn outputs O^T
#   B: token-parallel output-proj + FFN (512 tokens/core)
# Activations kept feature-major (features on partitions).
import sys
sys.path.insert(0, '/opt/trn_rl_repo')
import math
import numpy as np
import bass_rust
import concourse.bass as bass
import concourse.mybir as mybir
import concourse.tile as tile
from concourse import bacc

F32 = mybir.dt.float32
F32R = mybir.dt.float32r
BF16 = mybir.dt.bfloat16
AF = mybir.ActivationFunctionType

import ml_dtypes
NP_BF16 = ml_dtypes.bfloat16

E = 1024
H = 16
D = 64
HID = 4096
B = 2
S = 2048
NB = 32
MAXD = 128
LN_EPS = 1e-6
KQ_EPS = 1e-6
NCORE = 8
HPC = H // NCORE          # heads per core (2)
TPC = B * S // NCORE      # tokens per core (512)
NQC = S // 512            # q-chunks per batch (4)
BOFF = 1024               # bias vec offset: index j = d + BOFF
MASKVAL = -20.0


def rel_bucket(d):
    d = np.asarray(d)
    max_exact = NB // 2
    safe = np.maximum(d, 1).astype(np.float64)
    large = max_exact + (
        np.log(safe / max_exact) / math.log(MAXD / max_exact) * (NB - max_exact)
    ).astype(np.int32)
    large = np.minimum(large, NB - 1)
    return np.where(d < max_exact, d, large)


def build_launch_a():
    nc = bacc.Bacc("TRN2", target_bir_lowering=False, debug=False)
    xT_d = nc.dram_tensor("xT", [E, B * S], F32R, kind="ExternalInput").ap()
    wq_d = nc.dram_tensor("wq", [128, 8 * HPC * D], F32R, kind="ExternalInput").ap()
    wk_d = nc.dram_tensor("wk", [128, 8 * HPC * D], F32R, kind="ExternalInput").ap()
    wv_d = nc.dram_tensor("wv", [128, 8 * HPC * D], F32R, kind="ExternalInput").ap()
    biast_d = nc.dram_tensor("biast", [HPC * 5, 128, 512], F32,
                             kind="ExternalInput")
    fconst_d = nc.dram_tensor("fconst", [128, HPC], F32, kind="ExternalInput").ap()
    scale2_d = nc.dram_tensor("scale2", [HPC, 1], F32, kind="ExternalInput").ap()
    hsel_d = nc.dram_tensor("hsel", [HPC, 128], F32R, kind="ExternalInput").ap()
    onesc_d = nc.dram_tensor("onesc", [128, 1], F32R, kind="ExternalInput").ap()
    onesr_d = nc.dram_tensor("onesr", [1, 128], F32R, kind="ExternalInput").ap()
    ones64_d = nc.dram_tensor("ones64", [1, 64], F32R, kind="ExternalInput").ap()
    hsum_d = nc.dram_tensor("hsum", [128, HPC], F32R, kind="ExternalInput").ap()
    identc_d = nc.dram_tensor("identc", [128, 64], F32R, kind="ExternalInput").ap()
    oT_d = nc.dram_tensor("oT", [HPC * D, B * S], F32, kind="ExternalOutput").ap()

    with tile.TileContext(nc) as tc:
        with nc.allow_low_precision(reason="fp32r kernel"), \
             tc.tile_pool(name="const", bufs=1) as cpool, \
             tc.tile_pool(name="xt", bufs=1) as xpool, \
             tc.tile_pool(name="w", bufs=1) as wpool, \
             tc.tile_pool(name="qkv", bufs=1) as qkvpool, \
             tc.tile_pool(name="rawp", bufs=2) as rawpool, \
             tc.tile_pool(name="vtok", bufs=1) as vpool, \
             tc.tile_pool(name="ptile", bufs=4) as ppool, \
             tc.tile_pool(name="onorm", bufs=3) as opool, \
             tc.tile_pool(name="small", bufs=2) as spool, \
             tc.tile_pool(name="r1p", bufs=1) as rpool, \
             tc.tile_pool(name="ps_mm", bufs=4, space="PSUM") as ps_mm, \
             tc.tile_pool(name="ps_acc", bufs=2, space="PSUM") as ps_acc:

            # ---- constants ----
            fconst_t = cpool.tile([128, HPC], F32)
            nc.sync.dma_start(fconst_t[:], fconst_d[:])
            scale2_t = cpool.tile([HPC, 1], F32)
            nc.sync.dma_start(scale2_t[:], scale2_d[:])
            ones128 = cpool.tile([128, 1], F32R)
            nc.sync.dma_start(ones128[:], onesc_d[:])
            ones1x64 = cpool.tile([1, 64], F32R)
            nc.sync.dma_start(ones1x64[:], ones64_d[:])
            onesr = cpool.tile([1, 128], F32R)
            nc.sync.dma_start(onesr[:], onesr_d[:])
            hsum = cpool.tile([128, HPC], F32R)
            nc.sync.dma_start(hsum[:], hsum_d[:])
            # head-select [2, 128]: row h = 1 on cols h*64..
            hsel = cpool.tile([HPC, 128], F32R)
            nc.sync.dma_start(hsel[:], hsel_d[:])
            # stacked identities [128, 64] (f32r) for per-head PE transpose
            epsln_t = cpool.tile([128, 1], F32)
            nc.vector.memset(epsln_t[:], LN_EPS)
            epskq_t = cpool.tile([128, 1], F32)
            nc.vector.memset(epskq_t[:], KQ_EPS)
            ident = cpool.tile([128, 64], F32R)
            nc.sync.dma_start(ident[:], identc_d[:])
            # near-band bias tiles: [128, (h*5+di)*512 + f]
            biast = cpool.tile([128, HPC * 5 * 512], F32)
            nc.sync.dma_start(
                biast[:],
                bass_rust.AP(biast_d, 0, [[512, 128], [65536, HPC * 5],
                                          [1, 512]]))

            # weights (ln1-folded, f32r): [128, e-tile, col]
            wq_t = wpool.tile([128, 8, HPC * D], F32R, tag="wq")
            wk_t = wpool.tile([128, 8, HPC * D], F32R, tag="wk")
            wv_t = wpool.tile([128, 8, HPC * D], F32R, tag="wv")
            for (w_t, w_d) in ((wq_t, wq_d), (wk_t, wk_d), (wv_t, wv_d)):
                nc.sync.dma_start(
                    w_t[:], w_d.rearrange("p (i m) -> p i m", i=8))

            for b in range(B):
                # ---- xT e-tiles [128, 2048] ----
                xts = []
                for i in range(8):
                    xt = xpool.tile([128, S], F32R, tag=f"x{i}")
                    nc.sync.dma_start(xt[:], xT_d[i * 128:(i + 1) * 128,
                                                  b * S:(b + 1) * S])
                    xts.append(xt)

                # ---- r1 = 1/sqrt(mean(x^2)+eps) ----
                r1 = rpool.tile([1, S], F32R, tag="r1")
                for ch in range(S // 512):
                    csl = slice(ch * 512, (ch + 1) * 512)
                    ssx_ps = ps_acc.tile([128, 512], F32, tag="acc")
                    for i in range(8):
                        sq = spool.tile([128, 512], F32R, tag="sqx")
                        nc.scalar.activation(sq[:], xts[i][:, csl], AF.Square)
                        nc.tensor.matmul(ssx_ps[0:1, :], ones128[:], sq[:],
                                         start=(i == 0), stop=(i == 7))
                    r1sq = spool.tile([1, 512], F32, tag="r1sq")
                    nc.scalar.activation(r1sq[:], ssx_ps[0:1, :], AF.Sqrt,
                                         bias=epsln_t[0:1, :], scale=1.0 / E)
                    nc.vector.reciprocal(r1[:, csl], r1sq[:])

                # ---- projections: raw qT/kT/vT packed [128, 2048] ----
                def project(w_t, name):
                    pool_ = rawpool if name == "raw" else qkvpool
                    out = pool_.tile([128, S], F32R, tag=name)
                    for ch in range(S // 512):
                        csl = slice(ch * 512, (ch + 1) * 512)
                        ps = ps_mm.tile([128, 512], F32, tag="mm")
                        for i in range(8):
                            nc.tensor.matmul(
                                ps[:], w_t[:, i, :], xts[i][:, csl],
                                start=(i == 0), stop=(i == 7))
                        nc.scalar.activation(out[:, csl], ps[:], AF.Copy)
                    return out

                qT_raw = project(wq_t, "raw")
                kT_raw = project(wk_t, "raw")
                vT_raw = project(wv_t, "vT_raw")

                # ---- L2-normalize q (scale-folded) and k ----
                def l2norm(raw, name, fold_scale):
                    out = qkvpool.tile([128, S], F32R, tag=name)
                    for ch in range(S // 512):
                        csl = slice(ch * 512, (ch + 1) * 512)
                        sq = spool.tile([128, 512], F32R, tag="sqn")
                        nc.scalar.activation(sq[:], raw[:, csl], AF.Square)
                        ss_ps = ps_mm.tile([128, 512], F32, tag="mm")
                        nc.tensor.matmul(ss_ps[0:HPC, :], hsum[:], sq[:],
                                         start=True, stop=True)
                        rnsq = spool.tile([HPC, 512], F32, tag="rnsq")
                        nc.scalar.activation(rnsq[:], ss_ps[0:HPC, :], AF.Sqrt,
                                             bias=epskq_t[0:HPC, :])
                        rn = spool.tile([HPC, 512], F32R, tag="rn")
                        nc.vector.reciprocal(rn[:], rnsq[:])
                        if fold_scale:
                            nc.vector.tensor_scalar_mul(rn[:], rn[:],
                                                        scale2_t[:])
                        bc_ps = ps_mm.tile([128, 512], F32, tag="mm")
                        nc.tensor.matmul(bc_ps[:], hsel[:], rn[:],
                                         start=True, stop=True)
                        nc.vector.tensor_mul(out[:, csl], raw[:, csl], bc_ps[:])
                    return out

                qT = l2norm(qT_raw, "qT", True)
                kT = l2norm(kT_raw, "kT", False)

                # ---- v scaled by r1 in place (feature-major) ----
                vT_s = vT_raw
                mvv = spool.tile([128, 1], F32, tag="mvv")
                mv4 = spool.tile([128, NQC], F32, tag="mv4")
                for ch in range(NQC):
                    csl = slice(ch * 512, (ch + 1) * 512)
                    bc_ps = ps_mm.tile([128, 512], F32, tag="mm")
                    nc.tensor.matmul(bc_ps[:], onesr[:], r1[:, csl],
                                     start=True, stop=True)
                    nc.vector.tensor_mul(vT_s[:, csl], vT_raw[:, csl], bc_ps[:])
                    nc.vector.reduce_sum(mv4[:, ch:ch + 1], vT_s[:, csl],
                                         mybir.AxisListType.X)
                nc.vector.reduce_sum(mvv[:], mv4[:], mybir.AxisListType.X)

                v_augs = [[], []]
                for kt in range(S // 128):
                    for h in range(HPC):
                        va = vpool.tile([128, D + 1], F32R, tag=f"va{h}_{kt}")
                        nc.vector.tensor_copy(va[:, D:D + 1], ones128[:])
                        tr_ps = ps_mm.tile([128, 512], F32R, tag="mm")
                        nc.tensor.transpose(
                            tr_ps[0:128, 0:64],
                            vT_s[h * D:(h + 1) * D, kt * 128:(kt + 1) * 128],
                            ident[h * D:(h + 1) * D, :])
                        nc.vector.tensor_copy(va[:, 0:D], tr_ps[0:128, 0:64])
                        v_augs[h].append(va)

                # ---- attention ----
                for h in range(HPC):
                    hd = slice(h * D, (h + 1) * D)
                    for qc in range(NQC):
                        Q0 = qc * 512
                        qsl = slice(Q0, Q0 + 512)
                        nkt = (Q0 + 512) // 128
                        o_ps = ps_acc.tile([128, 512], F32, tag="acc")
                        for kt in range(nkt):
                            K0 = kt * 128
                            s_ps = ps_mm.tile([128, 512], F32, tag="mm")
                            nc.tensor.matmul(
                                s_ps[:], kT[hd, K0:K0 + 128], qT[hd, qsl],
                                start=True, stop=True)
                            p_t = ppool.tile([128, 512], F32R, tag="p")
                            if Q0 - K0 >= 255:
                                nc.scalar.activation(
                                    p_t[:], s_ps[:], AF.Exp,
                                    bias=fconst_t[:, h:h + 1])
                            else:
                                di = 1 + kt - 4 * qc
                                bsl = slice((h * 5 + di) * 512,
                                            (h * 5 + di + 1) * 512)
                                nc.vector.tensor_add(s_ps[:], s_ps[:],
                                                     biast[:, bsl])
                                nc.scalar.activation(p_t[:], s_ps[:], AF.Exp)
                            nc.tensor.matmul(
                                o_ps[0:D + 1, :], v_augs[h][kt][:], p_t[:],
                                start=(kt == 0), stop=(kt == nkt - 1))
                        srec = spool.tile([1, 512], F32R, tag="srec")
                        nc.vector.reciprocal(srec[:], o_ps[D:D + 1, :])
                        nb_ps = ps_mm.tile([128, 512], F32, tag="mm")
                        nc.tensor.matmul(nb_ps[0:D, :], ones1x64[:], srec[:],
                                         start=True, stop=True)
                        nb = spool.tile([64, 512], F32, tag="nbs")
                        nc.scalar.activation(nb[:], nb_ps[0:D, :], AF.Copy)
                        o_n = opool.tile([64, 512], F32, tag="on")
                        nc.vector.tensor_mul(o_n[:], o_ps[0:D, :], nb[:])
                        if qc == 0:
                            nc.scalar.activation(
                                o_n[:, 0:1], mvv[h * D:(h + 1) * D, :],
                                AF.Copy, scale=1.0 / S)
                        nc.sync.dma_start(
                            oT_d[hd, b * S + Q0: b * S + Q0 + 512], o_n[:])
    nc.compile()
    return nc


def prep_a_inputs(inputs):
    x = np.asarray(inputs["x"], np.float32)
    ln1 = np.asarray(inputs["ln1_w"], np.float32)
    Wq = np.asarray(inputs["Wq"], np.float32)
    Wk = np.asarray(inputs["Wk"], np.float32)
    Wv = np.asarray(inputs["Wv"], np.float32)
    rb = np.asarray(inputs["rel_bias"], np.float32)
    scale = np.asarray(inputs["scale"], np.float32)
    xT = np.ascontiguousarray(x.reshape(B * S, E).T)
    d = np.arange(2048) - BOFF
    bucket = rel_bucket(np.maximum(d, 1))
    biasv_all = np.where(
        (d < 1)[None, :], np.float32(MASKVAL),
        scale[:, None] * rb[bucket, :].T.astype(np.float32)).astype(np.float32)
    # bias delta-tiles: [H, 5, 128, 512]; B[h,di,p,f] = v_h[BOFF+128-128*di+f-p]
    di_ = np.arange(5)[:, None, None]
    p_ = np.arange(128)[None, :, None]
    f_ = np.arange(512)[None, None, :]
    idx = BOFF + 128 - 128 * di_ + f_ - p_
    biast_all = biasv_all[:, idx]  # [H, 5, 128, 512]
    fconst_all = (scale * rb[NB - 1, :]).astype(np.float32)

    def tile_w(w):  # [1024, M] -> [128, 8*M]
        M = w.shape[1]
        return np.ascontiguousarray(
            w.reshape(8, 128, M).transpose(1, 0, 2).reshape(128, 8 * M))
    in_maps = []
    for c in range(NCORE):
        hs = slice(c * HPC, (c + 1) * HPC)
        cs = slice(c * HPC * D, (c + 1) * HPC * D)
        hsel_np = np.zeros((HPC, 128), np.float32)
        for h in range(HPC):
            hsel_np[h, h * D:(h + 1) * D] = 1.0
        ident_np = np.concatenate([np.eye(D, dtype=np.float32)] * 2, axis=0)
        hsum_np = np.zeros((128, HPC), np.float32)
        for h in range(HPC):
            hsum_np[h * D:(h + 1) * D, h] = 1.0
        in_maps.append({
            "xT": xT,
            "hsel": hsel_np,
            "identc": ident_np,
            "onesc": np.ones((128, 1), np.float32),
            "onesr": np.ones((1, 128), np.float32),
            "ones64": np.ones((1, 64), np.float32),
            "hsum": hsum_np,
            "wq": tile_w(ln1[:, None] * Wq[:, cs]),
            "wk": tile_w(ln1[:, None] * Wk[:, cs]),
            "wv": tile_w(ln1[:, None] * Wv[:, cs]),
            "biast": np.ascontiguousarray(
                biast_all[hs].reshape(HPC * 5, 128, 512)),
            "fconst": np.ascontiguousarray(
                np.broadcast_to(fconst_all[hs], (128, HPC))),
            "scale2": np.ascontiguousarray(scale[hs, None]),
        })
    return in_maps


def build_launch_b():
    nc = bacc.Bacc("TRN2", target_bir_lowering=False, debug=False)
    oT_d = nc.dram_tensor("oTs", [E, TPC], BF16, kind="ExternalInput").ap()
    xT_d = nc.dram_tensor("xTs", [E, TPC], F32R, kind="ExternalInput").ap()
    wo_d = nc.dram_tensor("wo", [E // 128, 128, E], BF16, kind="ExternalInput")
    w1_d = nc.dram_tensor("w1", [HID // 128, 128, E], BF16, kind="ExternalInput")
    w2_d = nc.dram_tensor("w2", [E // 128, 128, HID], BF16, kind="ExternalInput")
    onesc_d = nc.dram_tensor("onesc", [128, 1], BF16, kind="ExternalInput").ap()
    onesr_d = nc.dram_tensor("onesr", [1, 128], F32, kind="ExternalInput").ap()
    out_d = nc.dram_tensor("outT", [E, TPC], F32, kind="ExternalOutput").ap()

    NE = E // 128    # 8 e-tiles
    NH = HID // 128  # 32 h-tiles

    with tile.TileContext(nc) as tc:
        with nc.allow_low_precision(reason="bf16 kernel"), \
             tc.tile_pool(name="const", bufs=1) as cpool, \
             tc.tile_pool(name="io", bufs=1) as iopool, \
             tc.tile_pool(name="y", bufs=1) as ypool, \
             tc.tile_pool(name="h", bufs=1) as hpool, \
             tc.tile_pool(name="w1s", bufs=2) as w1pool, \
             tc.tile_pool(name="w2s", bufs=2) as w2pool, \
             tc.tile_pool(name="small", bufs=2) as spool, \
             tc.tile_pool(name="ps_mm", bufs=4, space="PSUM") as ps_mm, \
             tc.tile_pool(name="ps_acc", bufs=2, space="PSUM") as ps_acc:

            ones128 = cpool.tile([128, 1], BF16)
            nc.sync.dma_start(ones128[:], onesc_d[:])
            epsln_t = cpool.tile([128, 1], F32)
            nc.vector.memset(epsln_t[:], LN_EPS)

            oTs, xTs = [], []
            for i in range(NE):
                ot = iopool.tile([128, TPC], BF16, tag=f"o{i}")
                nc.sync.dma_start(ot[:], oT_d[i * 128:(i + 1) * 128, :])
                oTs.append(ot)
                xt = iopool.tile([128, TPC], F32R, tag=f"xs{i}")
                nc.sync.dma_start(xt[:], xT_d[i * 128:(i + 1) * 128, :])
                xTs.append(xt)
            # ---- y^T = Wo^T @ O^T + x^T ----
            yTs, yBs = [], []
            for i in range(NE):
                wo_t = w1pool.tile([128, NE, 128], BF16, tag="w1")
                nc.sync.dma_start(
                    wo_t[:], bass_rust.AP(wo_d, i * 128 * E,
                                          [[E, 128], [1, NE * 128]]))
                ps = ps_acc.tile([128, TPC], F32, tag="acc")
                for j in range(NE):
                    nc.tensor.matmul(ps[:], wo_t[:, j, :],
                                     oTs[j][:], start=(j == 0), stop=(j == NE - 1))
                yt = ypool.tile([128, TPC], F32R, tag=f"y{i}")
                nc.vector.tensor_add(yt[:], ps[:], xTs[i][:])
                yTs.append(yt)
                yb = ypool.tile([128, TPC], BF16, tag=f"yb{i}")
                nc.scalar.activation(yb[:], yt[:], AF.Copy)
                yBs.append(yb)

            # ---- r2 = 1/sqrt(mean(y^2)+eps) ----
            ssy_ps = ps_acc.tile([128, TPC], F32, tag="acc")
            for i in range(NE):
                sq = spool.tile([128, TPC], BF16, tag="sqy")
                nc.vector.tensor_mul(sq[:], yBs[i][:], yBs[i][:])
                nc.tensor.matmul(ssy_ps[0:1, :], ones128[:], sq[:],
                                 start=(i == 0), stop=(i == NE - 1))
            r2sq = spool.tile([1, TPC], F32, tag="r2sq")
            nc.scalar.activation(r2sq[:], ssy_ps[0:1, :], AF.Sqrt,
                                 bias=epsln_t[0:1, :], scale=1.0 / E)
            r2 = spool.tile([1, TPC], F32, tag="r2")
            nc.vector.reciprocal_approx_fast(r2[:], r2sq[:])
            # broadcast r2 over 128 partitions
            r2b_ps = ps_mm.tile([128, TPC], F32, tag="mm")
            o1x128 = cpool.tile([1, 128], F32)
            nc.sync.dma_start(o1x128[:], onesr_d[:])
            nc.tensor.matmul(r2b_ps[:], o1x128[:], r2[:], start=True, stop=True)
            r2b = cpool.tile([128, TPC], F32)
            nc.scalar.activation(r2b[:], r2b_ps[:], AF.Copy)

            # ---- h^T = relu(W1'^T y^T) ----
            hts = []
            for ht in range(NH):
                w1_t = w1pool.tile([128, NE, 128], BF16, tag="w1")
                nc.sync.dma_start(
                    w1_t[:], bass_rust.AP(w1_d, ht * 128 * E,
                                          [[E, 128], [1, NE * 128]]))
                ps = ps_mm.tile([128, TPC], F32, tag="mm")
                for j in range(NE):
                    nc.tensor.matmul(ps[:], w1_t[:, j, :], yBs[j][:],
                                     start=(j == 0), stop=(j == NE - 1))
                h_t = hpool.tile([128, TPC], BF16, tag=f"h{ht}")
                nc.scalar.activation(h_t[:], ps[:], AF.Relu)
                hts.append(h_t)

            # ---- z = (h^T' W2)^T * r2 + y ----
            for i in range(NE):
                w2_t = w2pool.tile([128, NH, 128], BF16, tag="w2")
                nc.sync.dma_start(
                    w2_t[:], bass_rust.AP(w2_d, i * 128 * HID,
                                          [[HID, 128], [1, NH * 128]]))
                ps = ps_acc.tile([128, TPC], F32, tag="acc")
                for ht in range(NH):
                    nc.tensor.matmul(ps[:], w2_t[:, ht, :], hts[ht][:],
                                     start=(ht == 0), stop=(ht == NH - 1))
                zt = spool.tile([128, TPC], F32, tag="zt")
                nc.vector.tensor_mul(zt[:], ps[:], r2b[:])
                outt = spool.tile([128, TPC], F32, tag="outt")
                nc.vector.tensor_add(outt[:], zt[:], yTs[i][:])
                nc.sync.dma_start(out_d[i * 128:(i + 1) * 128, :], outt[:])
    nc.compile()
    return nc


def prep_b_inputs(inputs, oT_all):
    x = np.asarray(inputs["x"], np.float32)
    ln2 = np.asarray(inputs["ln2_w"], np.float32)
    def tile_cols(w):
        # [K, M] -> [M//128, 128, K]: out[i, p, j*128+m] = w[j*128+p, i*128+m]
        K, M = w.shape
        return np.ascontiguousarray(
            w.reshape(K // 128, 128, M // 128, 128)
            .transpose(2, 1, 0, 3).reshape(M // 128, 128, K))
    Wo = tile_cols(np.asarray(inputs["Wo"], np.float32)).astype(NP_BF16)
    W1 = tile_cols(
        ln2[:, None] * np.asarray(inputs["W1"], np.float32)).astype(NP_BF16)
    W2 = tile_cols(np.asarray(inputs["W2"], np.float32)).astype(NP_BF16)
    xT = x.reshape(B * S, E).T
    in_maps = []
    for c in range(NCORE):
        ts = slice(c * TPC, (c + 1) * TPC)
        in_maps.append({
            "oTs": np.ascontiguousarray(oT_all[:, ts]).astype(NP_BF16),
            "onesc": np.ones((128, 1), NP_BF16),
            "onesr": np.ones((1, 128), np.float32),
            "xTs": np.ascontiguousarray(xT[:, ts]),
            "wo": Wo, "w1": W1, "w2": W2,
        })
    return in_maps


_CACHE = {}


def _get_compiled():
    if "a" not in _CACHE:
        _CACHE["a"] = build_launch_a()
    if "b" not in _CACHE:
        _CACHE["b"] = build_launch_b()
    return _CACHE["a"], _CACHE["b"]


def kernel(**inputs):
    from concourse import bass_utils
    inputs = {k: np.asarray(v) for k, v in inputs.items()}
    nca, ncb = _get_compiled()
    in_maps_a = prep_a_inputs(inputs)
    res_a = bass_utils.run_bass_kernel_spmd(
        nca, in_maps_a, core_ids=list(range(NCORE)))
    oT_all = np.concatenate([res_a.results[c]["oT"] for c in range(NCORE)],
                            axis=0)  # [E, B*S], rows = h*64+d
    in_maps_b = prep_b_inputs(inputs, oT_all)
    res_b = bass_utils.run_bass_kernel_spmd(
        ncb, in_maps_b, core_ids=list(range(NCORE)))
    outT = np.concatenate([res_b.results[c]["outT"] for c in range(NCORE)],
                          axis=1)  # [E, B*S]
    return np.ascontiguousarray(outT.T).reshape(B, S, E).astype(np.float32)



# revision 16
# speedup vs baseline: 1.5954x; 1.3187x over previous
# Trainium2 Bass kernels for nn_DecoderLayer (T5-style decoder layer).
# Two SPMD launches over 8 cores:
#   A: head-parallel attention (2 heads/core) -> per-head attn outputs O^T
#   B: token-parallel output-proj + FFN (512 tokens/core)
# Activations kept feature-major (features on partitions).
import sys
sys.path.insert(0, '/opt/trn_rl_repo')
import math
import numpy as np
import bass_rust
import concourse.bass as bass
import concourse.mybir as mybir
import concourse.tile as tile
from concourse import bacc

F32 = mybir.dt.float32
F32R = mybir.dt.float32r
BF16 = mybir.dt.bfloat16
AF = mybir.ActivationFunctionType

import ml_dtypes
NP_BF16 = ml_dtypes.bfloat16

E = 1024
H = 16
D = 64
HID = 4096
B = 2
S = 2048
NB = 32
MAXD = 128
LN_EPS = 1e-6
KQ_EPS = 1e-6
NCORE = 8
HPC = H // NCORE          # heads per core (2)
TPC = B * S // NCORE      # tokens per core (512)
NQC = S // 512            # q-chunks per batch (4)
BOFF = 1024               # bias vec offset: index j = d + BOFF
MASKVAL = -20.0


def rel_bucket(d):
    d = np.asarray(d)
    max_exact = NB // 2
    safe = np.maximum(d, 1).astype(np.float64)
    large = max_exact + (
        np.log(safe / max_exact) / math.log(MAXD / max_exact) * (NB - max_exact)
    ).astype(np.int32)
    large = np.minimum(large, NB - 1)
    return np.where(d < max_exact, d, large)


def build_launch_a():
    nc = bacc.Bacc("TRN2", target_bir_lowering=False, debug=False)
    xT_d = nc.dram_tensor("xT", [E, B * S], BF16, kind="ExternalInput").ap()
    wq_d = nc.dram_tensor("wq", [128, 8 * HPC * D], BF16, kind="ExternalInput").ap()
    wk_d = nc.dram_tensor("wk", [128, 8 * HPC * D], BF16, kind="ExternalInput").ap()
    wv_d = nc.dram_tensor("wv", [128, 8 * HPC * D], BF16, kind="ExternalInput").ap()
    expb_d = nc.dram_tensor("expb", [HPC * 5, 128, 512], BF16,
                            kind="ExternalInput")
    fconst_d = nc.dram_tensor("fconst", [128, HPC], F32, kind="ExternalInput").ap()
    scaleb_d = nc.dram_tensor("scaleb", [128, HPC], F32, kind="ExternalInput").ap()
    onesc_d = nc.dram_tensor("onesc", [128, 1], BF16, kind="ExternalInput").ap()
    hsum_d = nc.dram_tensor("hsum", [128, HPC], BF16, kind="ExternalInput").ap()
    hsel_d = nc.dram_tensor("hsel", [HPC, 128], BF16, kind="ExternalInput").ap()
    identc_d = nc.dram_tensor("identc", [128, 64], BF16, kind="ExternalInput").ap()
    oT_d = nc.dram_tensor("oT", [HPC * D, B * S], BF16, kind="ExternalOutput").ap()

    with tile.TileContext(nc) as tc:
        with nc.allow_low_precision(reason="bf16 kernel"), \
             tc.tile_pool(name="const", bufs=1) as cpool, \
             tc.tile_pool(name="xt", bufs=2) as xpool, \
             tc.tile_pool(name="w", bufs=1) as wpool, \
             tc.tile_pool(name="qkv", bufs=2) as qkvpool, \
             tc.tile_pool(name="vtok", bufs=2) as vpool, \
             tc.tile_pool(name="ptile", bufs=4) as ppool, \
             tc.tile_pool(name="onorm", bufs=3) as opool, \
             tc.tile_pool(name="small", bufs=3) as spool, \
             tc.tile_pool(name="r1p", bufs=2) as rpool, \
             tc.tile_pool(name="ps_pair", bufs=2, space="PSUM") as ps_pair, \
             tc.tile_pool(name="ps_sm", bufs=2, space="PSUM") as ps_sm, \
             tc.tile_pool(name="ps_acc", bufs=2, space="PSUM") as ps_acc:

            # ---- constants ----
            fconst_t = cpool.tile([128, HPC], F32)
            nc.sync.dma_start(fconst_t[:], fconst_d[:])
            scaleb_t = cpool.tile([128, HPC], F32)
            nc.sync.dma_start(scaleb_t[:], scaleb_d[:])
            ones128 = cpool.tile([128, 1], BF16)
            nc.sync.dma_start(ones128[:], onesc_d[:])
            hsum = cpool.tile([128, HPC], BF16)
            nc.sync.dma_start(hsum[:], hsum_d[:])
            hsel = cpool.tile([HPC, 128], BF16)
            nc.sync.dma_start(hsel[:], hsel_d[:])
            epsln_t = cpool.tile([128, 1], F32)
            nc.vector.memset(epsln_t[:], LN_EPS)
            epskq_t = cpool.tile([128, 1], F32)
            nc.vector.memset(epskq_t[:], KQ_EPS)
            # stacked identities [128, 64] for per-head PE transpose
            ident = cpool.tile([128, 64], BF16)
            nc.sync.dma_start(ident[:], identc_d[:])
            # near-band exp(bias - fconst) tiles: [128, (h*5+di)*512 + f]
            expb = cpool.tile([128, HPC * 5 * 512], BF16)
            nc.sync.dma_start(
                expb[:],
                bass_rust.AP(expb_d, 0, [[512, 128], [65536, HPC * 5],
                                         [1, 512]]))

            # weights (ln1-folded, bf16): [128, e-tile, col]
            wq_t = wpool.tile([128, 8, HPC * D], BF16, tag="wq")
            wk_t = wpool.tile([128, 8, HPC * D], BF16, tag="wk")
            wv_t = wpool.tile([128, 8, HPC * D], BF16, tag="wv")
            for (w_t, w_d) in ((wq_t, wq_d), (wk_t, wk_d), (wv_t, wv_d)):
                nc.sync.dma_start(
                    w_t[:], w_d.rearrange("p (i m) -> p i m", i=8))

            for b in range(B):
                # ---- xT e-tiles [128, 2048] ----
                xts = []
                for i in range(8):
                    xt = xpool.tile([128, S], BF16, tag=f"x{i}")
                    nc.sync.dma_start(xt[:], xT_d[i * 128:(i + 1) * 128,
                                                  b * S:(b + 1) * S])
                    xts.append(xt)

                # ---- r1 = 1/sqrt(mean(x^2)+eps) ----
                r1 = rpool.tile([1, S], F32, tag="r1")
                for ch in range(S // 512):
                    csl = slice(ch * 512, (ch + 1) * 512)
                    ssx_ps = ps_sm.tile([128, 512], F32, tag="sm")
                    for i in range(8):
                        sq = spool.tile([128, 512], BF16, tag="sqx")
                        nc.vector.tensor_mul(sq[:], xts[i][:, csl],
                                             xts[i][:, csl])
                        nc.tensor.matmul(ssx_ps[0:1, :], ones128[:], sq[:],
                                         start=(i == 0), stop=(i == 7))
                    r1sq = spool.tile([1, 512], F32, tag="r1sq")
                    nc.scalar.activation(r1sq[:], ssx_ps[0:1, :], AF.Sqrt,
                                         bias=epsln_t[0:1, :], scale=1.0 / E)
                    nc.vector.reciprocal_approx_fast(r1[:, csl], r1sq[:])

                # ---- projections: raw qT/kT/vT packed [128, 2048] ----
                # i-outer loop reuses the stationary weight tile across the
                # 4 token chunks (2 psum pair-tiles of 2 chunks each).
                def project(w_t, name, copy_eng):
                    out = qkvpool.tile([128, S], BF16, tag=name)
                    pp0 = ps_pair.tile([128, 1024], F32, tag="pair")
                    pp1 = ps_pair.tile([128, 1024], F32, tag="pair")
                    pps = (pp0, pp1)
                    for i in range(8):
                        for ch in range(4):
                            csl = slice(ch * 512, (ch + 1) * 512)
                            pp = pps[ch // 2]
                            off = (ch % 2) * 512
                            nc.tensor.matmul(
                                pp[:, off:off + 512], w_t[:, i, :],
                                xts[i][:, csl], start=(i == 0), stop=(i == 7))
                    for j in range(2):
                        osl = slice(j * 1024, (j + 1) * 1024)
                        if copy_eng == "scalar":
                            nc.scalar.activation(out[:, osl], pps[j][:],
                                                 AF.Copy)
                        else:
                            nc.vector.tensor_copy(out[:, osl], pps[j][:])
                    return out

                qT = project(wq_t, "qT", "scalar")
                kT = project(wk_t, "kT", "vector")
                vT_s = project(wv_t, "vT", "scalar")

                # ---- L2-normalize q and k in place ----
                def l2norm(raw):
                    for ch in range(S // 512):
                        csl = slice(ch * 512, (ch + 1) * 512)
                        sq = spool.tile([128, 512], BF16, tag="sqn")
                        nc.vector.tensor_mul(sq[:], raw[:, csl], raw[:, csl])
                        ss_ps = ps_sm.tile([128, 512], F32, tag="sm")
                        nc.tensor.matmul(ss_ps[0:HPC, :], hsum[:], sq[:],
                                         start=True, stop=True)
                        rnsq = spool.tile([HPC, 512], F32, tag="rnsq")
                        nc.scalar.activation(rnsq[:], ss_ps[0:HPC, :], AF.Sqrt,
                                             bias=epskq_t[0:HPC, :])
                        rn = spool.tile([HPC, 512], F32, tag="rn")
                        nc.vector.reciprocal_approx_fast(rn[:], rnsq[:])
                        rnb = spool.tile([HPC, 512], BF16, tag="rnb")
                        nc.gpsimd.tensor_copy(rnb[:], rn[:])
                        bc_ps = ps_sm.tile([128, 512], F32, tag="sm")
                        nc.tensor.matmul(bc_ps[:], hsel[:], rnb[:],
                                         start=True, stop=True)
                        nc.vector.tensor_mul(raw[:, csl], raw[:, csl],
                                             bc_ps[:])

                l2norm(qT)
                l2norm(kT)

                # ---- v scaled by r1 in place (feature-major) ----
                mvv = spool.tile([128, 1], F32, tag="mvv")
                mv4 = spool.tile([128, NQC], F32, tag="mv4")
                for ch in range(NQC):
                    csl = slice(ch * 512, (ch + 1) * 512)
                    bc = rpool.tile([128, 512], F32, tag="bc")
                    nc.gpsimd.partition_broadcast(bc[:], r1[:, csl],
                                                  channels=128)
                    nc.vector.tensor_mul(vT_s[:, csl], vT_s[:, csl], bc[:])
                    nc.vector.reduce_sum(mv4[:, ch:ch + 1], vT_s[:, csl],
                                         mybir.AxisListType.X)
                nc.vector.reduce_sum(mvv[:], mv4[:], mybir.AxisListType.X)

                v_augs = [[], []]
                for kt in range(S // 128):
                    for h in range(HPC):
                        va = vpool.tile([128, D + 1], BF16, tag=f"va{h}_{kt}")
                        nc.vector.tensor_copy(va[:, D:D + 1], ones128[:])
                        tr_ps = ps_sm.tile([128, 64], BF16, tag="sm")
                        nc.tensor.transpose(
                            tr_ps[0:128, 0:64],
                            vT_s[h * D:(h + 1) * D, kt * 128:(kt + 1) * 128],
                            ident[h * D:(h + 1) * D, :])
                        nc.vector.tensor_copy(va[:, 0:D], tr_ps[0:128, 0:64])
                        v_augs[h].append(va)

                # ---- attention ----
                # exp over paired k-tiles: p = exp(s*scale_h + fconst_h),
                # near-band tiles then multiplied by exp(bias - fconst).
                for h in range(HPC):
                    hd = slice(h * D, (h + 1) * D)
                    for qc in range(NQC):
                        Q0 = qc * 512
                        qsl = slice(Q0, Q0 + 512)
                        nkt = (Q0 + 512) // 128
                        o_ps = ps_acc.tile([128, 512], F32, tag="acc")
                        for pr in range(nkt // 2):
                            pp = ps_pair.tile([128, 1024], F32, tag="pair")
                            for t2 in range(2):
                                K0 = (2 * pr + t2) * 128
                                nc.tensor.matmul(
                                    pp[:, t2 * 512:(t2 + 1) * 512],
                                    kT[hd, K0:K0 + 128], qT[hd, qsl],
                                    start=True, stop=True)
                            p_t = ppool.tile([128, 1024], BF16, tag="p")
                            nc.scalar.activation(
                                p_t[:], pp[:], AF.Exp,
                                bias=fconst_t[:, h:h + 1],
                                scale=scaleb_t[:, h:h + 1])
                            for t2 in range(2):
                                kt = 2 * pr + t2
                                psl = slice(t2 * 512, (t2 + 1) * 512)
                                di = 1 + kt - 4 * qc
                                if 0 <= di <= 4:
                                    bsl = slice((h * 5 + di) * 512,
                                                (h * 5 + di + 1) * 512)
                                    nc.vector.tensor_mul(
                                        p_t[:, psl], p_t[:, psl],
                                        expb[:, bsl])
                                nc.tensor.matmul(
                                    o_ps[0:D + 1, :], v_augs[h][kt][:],
                                    p_t[:, psl],
                                    start=(kt == 0), stop=(kt == nkt - 1))
                        den = spool.tile([1, 512], F32, tag="den")
                        nc.scalar.activation(den[:], o_ps[D:D + 1, :], AF.Copy)
                        srec = spool.tile([1, 512], F32, tag="srec")
                        nc.vector.reciprocal_approx_fast(srec[:], den[:])
                        nb = rpool.tile([64, 512], F32, tag="nb")
                        nc.gpsimd.partition_broadcast(nb[:], srec[:],
                                                      channels=D)
                        o_n = opool.tile([64, 512], BF16, tag="on")
                        nc.vector.tensor_mul(o_n[:], o_ps[0:D, :], nb[:])
                        if qc == 0:
                            nc.scalar.activation(
                                o_n[:, 0:1], mvv[h * D:(h + 1) * D, :],
                                AF.Copy, scale=1.0 / S)
                        nc.sync.dma_start(
                            oT_d[hd, b * S + Q0: b * S + Q0 + 512], o_n[:])
    nc.compile()
    return nc


def prep_a_inputs(inputs):
    x = np.asarray(inputs["x"], np.float32)
    ln1 = np.asarray(inputs["ln1_w"], np.float32)
    Wq = np.asarray(inputs["Wq"], np.float32)
    Wk = np.asarray(inputs["Wk"], np.float32)
    Wv = np.asarray(inputs["Wv"], np.float32)
    rb = np.asarray(inputs["rel_bias"], np.float32)
    scale = np.asarray(inputs["scale"], np.float32)
    xT = np.ascontiguousarray(x.reshape(B * S, E).T).astype(NP_BF16)
    d = np.arange(2048) - BOFF
    bucket = rel_bucket(np.maximum(d, 1))
    biasv_all = np.where(
        (d < 1)[None, :], np.float32(MASKVAL),
        scale[:, None] * rb[bucket, :].T.astype(np.float32)).astype(np.float32)
    fconst_all = (scale * rb[NB - 1, :]).astype(np.float32)
    # exp(bias - fconst) delta-tiles: [H, 5, 128, 512];
    # E[h,di,p,f] = exp(v_h[BOFF+128-128*di+f-p] - fconst_h)
    expv_all = np.exp(biasv_all - fconst_all[:, None]).astype(np.float32)
    di_ = np.arange(5)[:, None, None]
    p_ = np.arange(128)[None, :, None]
    f_ = np.arange(512)[None, None, :]
    idx = BOFF + 128 - 128 * di_ + f_ - p_
    expb_all = expv_all[:, idx].astype(NP_BF16)  # [H, 5, 128, 512]

    def tile_w(w):  # [1024, M] -> [128, 8*M]
        M = w.shape[1]
        return np.ascontiguousarray(
            w.reshape(8, 128, M).transpose(1, 0, 2).reshape(128, 8 * M))
    ident_np = np.concatenate([np.eye(D, dtype=np.float32)] * 2,
                              axis=0).astype(NP_BF16)
    in_maps = []
    for c in range(NCORE):
        hs = slice(c * HPC, (c + 1) * HPC)
        cs = slice(c * HPC * D, (c + 1) * HPC * D)
        hsum_np = np.zeros((128, HPC), np.float32)
        for h in range(HPC):
            hsum_np[h * D:(h + 1) * D, h] = 1.0
        hsel_np = np.zeros((HPC, 128), np.float32)
        for h in range(HPC):
            hsel_np[h, h * D:(h + 1) * D] = 1.0
        in_maps.append({
            "xT": xT,
            "hsel": hsel_np.astype(NP_BF16),
            "identc": ident_np,
            "onesc": np.ones((128, 1), NP_BF16),
            "hsum": hsum_np.astype(NP_BF16),
            "wq": tile_w(ln1[:, None] * Wq[:, cs]).astype(NP_BF16),
            "wk": tile_w(ln1[:, None] * Wk[:, cs]).astype(NP_BF16),
            "wv": tile_w(ln1[:, None] * Wv[:, cs]).astype(NP_BF16),
            "expb": np.ascontiguousarray(
                expb_all[hs].reshape(HPC * 5, 128, 512)),
            "fconst": np.ascontiguousarray(
                np.broadcast_to(fconst_all[hs], (128, HPC))),
            "scaleb": np.ascontiguousarray(
                np.broadcast_to(scale[hs], (128, HPC))),
        })
    return in_maps


def build_launch_b():
    nc = bacc.Bacc("TRN2", target_bir_lowering=False, debug=False)
    oT_d = nc.dram_tensor("oTs", [E, TPC], BF16, kind="ExternalInput").ap()
    xT_d = nc.dram_tensor("xTs", [E, TPC], F32R, kind="ExternalInput").ap()
    wo_d = nc.dram_tensor("wo", [E // 128, 128, E], BF16, kind="ExternalInput")
    w1_d = nc.dram_tensor("w1", [HID // 128, 128, E], BF16, kind="ExternalInput")
    w2_d = nc.dram_tensor("w2", [E // 128, 128, HID], BF16, kind="ExternalInput")
    onesc_d = nc.dram_tensor("onesc", [128, 1], BF16, kind="ExternalInput").ap()
    onesr_d = nc.dram_tensor("onesr", [1, 128], F32, kind="ExternalInput").ap()
    out_d = nc.dram_tensor("outT", [E, TPC], F32, kind="ExternalOutput").ap()

    NE = E // 128    # 8 e-tiles
    NH = HID // 128  # 32 h-tiles

    with tile.TileContext(nc) as tc:
        with nc.allow_low_precision(reason="bf16 kernel"), \
             tc.tile_pool(name="const", bufs=1) as cpool, \
             tc.tile_pool(name="io", bufs=1) as iopool, \
             tc.tile_pool(name="y", bufs=1) as ypool, \
             tc.tile_pool(name="h", bufs=1) as hpool, \
             tc.tile_pool(name="w1s", bufs=2) as w1pool, \
             tc.tile_pool(name="w2s", bufs=2) as w2pool, \
             tc.tile_pool(name="small", bufs=2) as spool, \
             tc.tile_pool(name="ps_mm", bufs=4, space="PSUM") as ps_mm, \
             tc.tile_pool(name="ps_acc", bufs=2, space="PSUM") as ps_acc:

            ones128 = cpool.tile([128, 1], BF16)
            nc.sync.dma_start(ones128[:], onesc_d[:])
            epsln_t = cpool.tile([128, 1], F32)
            nc.vector.memset(epsln_t[:], LN_EPS)

            oTs, xTs = [], []
            for i in range(NE):
                ot = iopool.tile([128, TPC], BF16, tag=f"o{i}")
                nc.sync.dma_start(ot[:], oT_d[i * 128:(i + 1) * 128, :])
                oTs.append(ot)
                xt = iopool.tile([128, TPC], F32R, tag=f"xs{i}")
                nc.sync.dma_start(xt[:], xT_d[i * 128:(i + 1) * 128, :])
                xTs.append(xt)
            # ---- y^T = Wo^T @ O^T + x^T ----
            yTs, yBs = [], []
            for i in range(NE):
                wo_t = w1pool.tile([128, NE, 128], BF16, tag="w1")
                nc.sync.dma_start(
                    wo_t[:], bass_rust.AP(wo_d, i * 128 * E,
                                          [[E, 128], [1, NE * 128]]))
                ps = ps_acc.tile([128, TPC], F32, tag="acc")
                for j in range(NE):
                    nc.tensor.matmul(ps[:], wo_t[:, j, :],
                                     oTs[j][:], start=(j == 0), stop=(j == NE - 1))
                yt = ypool.tile([128, TPC], F32R, tag=f"y{i}")
                nc.vector.tensor_add(yt[:], ps[:], xTs[i][:])
                yTs.append(yt)
                yb = ypool.tile([128, TPC], BF16, tag=f"yb{i}")
                nc.scalar.activation(yb[:], yt[:], AF.Copy)
                yBs.append(yb)

            # ---- r2 = 1/sqrt(mean(y^2)+eps) ----
            ssy_ps = ps_acc.tile([128, TPC], F32, tag="acc")
            for i in range(NE):
                sq = spool.tile([128, TPC], BF16, tag="sqy")
                nc.vector.tensor_mul(sq[:], yBs[i][:], yBs[i][:])
                nc.tensor.matmul(ssy_ps[0:1, :], ones128[:], sq[:],
                                 start=(i == 0), stop=(i == NE - 1))
            r2sq = spool.tile([1, TPC], F32, tag="r2sq")
            nc.scalar.activation(r2sq[:], ssy_ps[0:1, :], AF.Sqrt,
                                 bias=epsln_t[0:1, :], scale=1.0 / E)
            r2 = spool.tile([1, TPC], F32, tag="r2")
            nc.vector.reciprocal_approx_fast(r2[:], r2sq[:])
            # broadcast r2 over 128 partitions
            r2b_ps = ps_mm.tile([128, TPC], F32, tag="mm")
            o1x128 = cpool.tile([1, 128], F32)
            nc.sync.dma_start(o1x128[:], onesr_d[:])
            nc.tensor.matmul(r2b_ps[:], o1x128[:], r2[:], start=True, stop=True)
            r2b = cpool.tile([128, TPC], F32)
            nc.scalar.activation(r2b[:], r2b_ps[:], AF.Copy)

            # ---- h^T = relu(W1'^T y^T) ----
            hts = []
            for ht in range(NH):
                w1_t = w1pool.tile([128, NE, 128], BF16, tag="w1")
                nc.sync.dma_start(
                    w1_t[:], bass_rust.AP(w1_d, ht * 128 * E,
                                          [[E, 128], [1, NE * 128]]))
                ps = ps_mm.tile([128, TPC], F32, tag="mm")
                for j in range(NE):
                    nc.tensor.matmul(ps[:], w1_t[:, j, :], yBs[j][:],
                                     start=(j == 0), stop=(j == NE - 1))
                h_t = hpool.tile([128, TPC], BF16, tag=f"h{ht}")
                nc.scalar.activation(h_t[:], ps[:], AF.Relu)
                hts.append(h_t)

            # ---- z = (h^T' W2)^T * r2 + y ----
            for i in range(NE):
                w2_t = w2pool.tile([128, NH, 128], BF16, tag="w2")
                nc.sync.dma_start(
                    w2_t[:], bass_rust.AP(w2_d, i * 128 * HID,
                                          [[HID, 128], [1, NH * 128]]))
                ps = ps_acc.tile([128, TPC], F32, tag="acc")
                for ht in range(NH):
                    nc.tensor.matmul(ps[:], w2_t[:, ht, :], hts[ht][:],
                                     start=(ht == 0), stop=(ht == NH - 1))
                zt = spool.tile([128, TPC], F32, tag="zt")
                nc.vector.tensor_mul(zt[:], ps[:], r2b[:])
                outt = spool.tile([128, TPC], F32, tag="outt")
                nc.vector.tensor_add(outt[:], zt[:], yTs[i][:])
                nc.sync.dma_start(out_d[i * 128:(i + 1) * 128, :], outt[:])
    nc.compile()
    return nc


def prep_b_inputs(inputs, oT_all):
    x = np.asarray(inputs["x"], np.float32)
    ln2 = np.asarray(inputs["ln2_w"], np.float32)
    def tile_cols(w):
        # [K, M] -> [M//128, 128, K]: out[i, p, j*128+m] = w[j*128+p, i*128+m]
        K, M = w.shape
        return np.ascontiguousarray(
            w.reshape(K // 128, 128, M // 128, 128)
            .transpose(2, 1, 0, 3).reshape(M // 128, 128, K))
    Wo = tile_cols(np.asarray(inputs["Wo"], np.float32)).astype(NP_BF16)
    W1 = tile_cols(
        ln2[:, None] * np.asarray(inputs["W1"], np.float32)).astype(NP_BF16)
    W2 = tile_cols(np.asarray(inputs["W2"], np.float32)).astype(NP_BF16)
    xT = x.reshape(B * S, E).T
    in_maps = []
    for c in range(NCORE):
        ts = slice(c * TPC, (c + 1) * TPC)
        in_maps.append({
            "oTs": np.ascontiguousarray(oT_all[:, ts]).astype(NP_BF16),
            "onesc": np.ones((128, 1), NP_BF16),
            "onesr": np.ones((1, 128), np.float32),
            "xTs": np.ascontiguousarray(xT[:, ts]),
            "wo": Wo, "w1": W1, "w2": W2,
        })
    return in_maps


_CACHE = {}


def _get_compiled():
    if "a" not in _CACHE:
        _CACHE["a"] = build_launch_a()
    if "b" not in _CACHE:
        _CACHE["b"] = build_launch_b()
    return _CACHE["a"], _CACHE["b"]


def kernel(**inputs):
    from concourse import bass_utils
    inputs = {k: np.asarray(v) for k, v in inputs.items()}
    nca, ncb = _get_compiled()
    in_maps_a = prep_a_inputs(inputs)
    res_a = bass_utils.run_bass_kernel_spmd(
        nca, in_maps_a, core_ids=list(range(NCORE)))
    oT_all = np.concatenate([res_a.results[c]["oT"] for c in range(NCORE)],
                            axis=0)  # [E, B*S], rows = h*64+d
    in_maps_b = prep_b_inputs(inputs, oT_all)
    res_b = bass_utils.run_bass_kernel_spmd(
        ncb, in_maps_b, core_ids=list(range(NCORE)))
    outT = np.concatenate([res_b.results[c]["outT"] for c in range(NCORE)],
                          axis=1)  # [E, B*S]
    return np.ascontiguousarray(outT.T).reshape(B, S, E).astype(np.float32)



# revision 21
# speedup vs baseline: 1.7658x; 1.1068x over previous
# Trainium2 Bass kernels for nn_DecoderLayer (T5-style decoder layer).
# Two SPMD launches over 8 cores:
#   A: head-parallel attention (2 heads/core) -> per-head attn outputs O^T
#   B: token-parallel output-proj + FFN (512 tokens/core)
# Activations kept feature-major (features on partitions).
import sys
sys.path.insert(0, '/opt/trn_rl_repo')
import math
import numpy as np
import bass_rust
import concourse.bass as bass
import concourse.mybir as mybir
import concourse.tile as tile
from concourse import bacc

F32 = mybir.dt.float32
F32R = mybir.dt.float32r
BF16 = mybir.dt.bfloat16
AF = mybir.ActivationFunctionType

import ml_dtypes
NP_BF16 = ml_dtypes.bfloat16

E = 1024
H = 16
D = 64
HID = 4096
B = 2
S = 2048
NB = 32
MAXD = 128
LN_EPS = 1e-6
KQ_EPS = 1e-6
NCORE = 8
HPC = H // NCORE          # heads per core (2)
TPC = B * S // NCORE      # tokens per core (512)
NQC = S // 512            # q-chunks per batch (4)
BOFF = 1024               # bias vec offset: index j = d + BOFF
MASKVAL = -20.0


def rel_bucket(d):
    d = np.asarray(d)
    max_exact = NB // 2
    safe = np.maximum(d, 1).astype(np.float64)
    large = max_exact + (
        np.log(safe / max_exact) / math.log(MAXD / max_exact) * (NB - max_exact)
    ).astype(np.int32)
    large = np.minimum(large, NB - 1)
    return np.where(d < max_exact, d, large)


def build_launch_a():
    nc = bacc.Bacc("TRN2", target_bir_lowering=False, debug=False)
    xT_d = nc.dram_tensor("xT", [E, B * S], BF16, kind="ExternalInput").ap()
    wq_d = nc.dram_tensor("wq", [128, 8 * HPC * D], BF16, kind="ExternalInput").ap()
    wk_d = nc.dram_tensor("wk", [128, 8 * HPC * D], BF16, kind="ExternalInput").ap()
    wv_d = nc.dram_tensor("wv", [128, 8 * HPC * D], BF16, kind="ExternalInput").ap()
    expb_d = nc.dram_tensor("expb", [HPC * 5, 128, 512], BF16,
                            kind="ExternalInput")
    fconst_d = nc.dram_tensor("fconst", [128, HPC], F32, kind="ExternalInput").ap()
    scaleb_d = nc.dram_tensor("scaleb", [128, HPC], F32, kind="ExternalInput").ap()
    onesc_d = nc.dram_tensor("onesc", [128, 1], BF16, kind="ExternalInput").ap()
    hsum_d = nc.dram_tensor("hsum", [128, HPC], BF16, kind="ExternalInput").ap()
    hsel_d = nc.dram_tensor("hsel", [HPC, 128], BF16, kind="ExternalInput").ap()
    identc_d = nc.dram_tensor("identc", [128, 64], BF16, kind="ExternalInput").ap()
    oT_d = nc.dram_tensor("oT", [HPC * D, B * S], BF16, kind="ExternalOutput").ap()

    with tile.TileContext(nc) as tc:
        with nc.allow_low_precision(reason="bf16 kernel"), \
             tc.tile_pool(name="const", bufs=1) as cpool, \
             tc.tile_pool(name="xt", bufs=2) as xpool, \
             tc.tile_pool(name="w", bufs=1) as wpool, \
             tc.tile_pool(name="qkv", bufs=2) as qkvpool, \
             tc.tile_pool(name="vtok", bufs=2) as vpool, \
             tc.tile_pool(name="ptile", bufs=4) as ppool, \
             tc.tile_pool(name="onorm", bufs=3) as opool, \
             tc.tile_pool(name="small", bufs=3) as spool, \
             tc.tile_pool(name="r1p", bufs=2) as rpool, \
             tc.tile_pool(name="ps_pair", bufs=2, space="PSUM") as ps_pair, \
             tc.tile_pool(name="ps_sm", bufs=2, space="PSUM") as ps_sm, \
             tc.tile_pool(name="ps_acc", bufs=2, space="PSUM") as ps_acc:

            # ---- constants ----
            fconst_t = cpool.tile([128, HPC], F32)
            nc.sync.dma_start(fconst_t[:], fconst_d[:])
            scaleb_t = cpool.tile([128, HPC], F32)
            nc.sync.dma_start(scaleb_t[:], scaleb_d[:])
            ones128 = cpool.tile([128, 1], BF16)
            nc.sync.dma_start(ones128[:], onesc_d[:])
            hsum = cpool.tile([128, HPC], BF16)
            nc.sync.dma_start(hsum[:], hsum_d[:])
            hsel = cpool.tile([HPC, 128], BF16)
            nc.sync.dma_start(hsel[:], hsel_d[:])
            epsln_t = cpool.tile([128, 1], F32)
            nc.vector.memset(epsln_t[:], LN_EPS)
            epskq_t = cpool.tile([128, 1], F32)
            nc.vector.memset(epskq_t[:], KQ_EPS)
            # stacked identities [128, 64] for per-head PE transpose
            ident = cpool.tile([128, 64], BF16)
            nc.sync.dma_start(ident[:], identc_d[:])
            # near-band exp(bias - fconst) tiles: [128, (h*5+di)*512 + f]
            expb = cpool.tile([128, HPC * 5 * 512], BF16)
            nc.sync.dma_start(
                expb[:],
                bass_rust.AP(expb_d, 0, [[512, 128], [65536, HPC * 5],
                                         [1, 512]]))

            # weights (ln1-folded, bf16): [128, e-tile, col]
            wq_t = wpool.tile([128, 8, HPC * D], BF16, tag="wq")
            wk_t = wpool.tile([128, 8, HPC * D], BF16, tag="wk")
            wv_t = wpool.tile([128, 8, HPC * D], BF16, tag="wv")
            for (w_t, w_d) in ((wq_t, wq_d), (wk_t, wk_d), (wv_t, wv_d)):
                nc.sync.dma_start(
                    w_t[:], w_d.rearrange("p (i m) -> p i m", i=8))

            for b in range(B):
                # ---- xT e-tiles [128, 2048] ----
                xts = []
                for i in range(8):
                    xt = xpool.tile([128, S], BF16, tag=f"x{i}")
                    nc.sync.dma_start(xt[:], xT_d[i * 128:(i + 1) * 128,
                                                  b * S:(b + 1) * S])
                    xts.append(xt)

                # ---- r1 = 1/sqrt(mean(x^2)+eps) ----
                r1 = rpool.tile([1, S], F32, tag="r1")
                for ch in range(S // 512):
                    csl = slice(ch * 512, (ch + 1) * 512)
                    # per-etile squares, tree-added on DVE; single matmul
                    # reduces the 128 partitions.
                    ssq = spool.tile([128, 512], BF16, tag="ssq")
                    nc.vector.tensor_mul(ssq[:], xts[0][:, csl],
                                         xts[0][:, csl])
                    for i in range(1, 8):
                        sq = spool.tile([128, 512], BF16, tag="sqx")
                        nc.vector.tensor_mul(sq[:], xts[i][:, csl],
                                             xts[i][:, csl])
                        nc.vector.tensor_add(ssq[:], ssq[:], sq[:])
                    ssx_ps = ps_sm.tile([128, 512], F32, tag="sm")
                    nc.tensor.matmul(ssx_ps[0:1, :], ones128[:], ssq[:],
                                     start=True, stop=True)
                    r1sq = spool.tile([1, 512], F32, tag="r1sq")
                    nc.scalar.activation(r1sq[:], ssx_ps[0:1, :], AF.Sqrt,
                                         bias=epsln_t[0:1, :], scale=1.0 / E)
                    nc.vector.reciprocal_approx_fast(r1[:, csl], r1sq[:])

                # ---- projections: raw qT/kT/vT packed [128, 2048] ----
                # i-outer loop reuses the stationary weight tile across the
                # 4 token chunks (2 psum pair-tiles of 2 chunks each).
                def project(w_t, name, copy_eng):
                    out = qkvpool.tile([128, S], BF16, tag=name)
                    pp0 = ps_pair.tile([128, 1024], F32, tag="pair")
                    pp1 = ps_pair.tile([128, 1024], F32, tag="pair")
                    pps = (pp0, pp1)
                    for i in range(8):
                        for ch in range(4):
                            csl = slice(ch * 512, (ch + 1) * 512)
                            pp = pps[ch // 2]
                            off = (ch % 2) * 512
                            nc.tensor.matmul(
                                pp[:, off:off + 512], w_t[:, i, :],
                                xts[i][:, csl], start=(i == 0), stop=(i == 7))
                    for j in range(2):
                        osl = slice(j * 1024, (j + 1) * 1024)
                        if copy_eng == "scalar":
                            nc.scalar.activation(out[:, osl], pps[j][:],
                                                 AF.Copy)
                        else:
                            nc.vector.tensor_copy(out[:, osl], pps[j][:])
                    return out

                qT = project(wq_t, "qT", "scalar")
                kT = project(wk_t, "kT", "vector")
                vT_s = project(wv_t, "vT", "vector")

                # ---- L2-normalize q and k in place ----
                def l2norm(raw):
                    for ch in range(S // 512):
                        csl = slice(ch * 512, (ch + 1) * 512)
                        sq = spool.tile([128, 512], BF16, tag="sqn")
                        nc.vector.tensor_mul(sq[:], raw[:, csl], raw[:, csl])
                        ss_ps = ps_sm.tile([128, 512], F32, tag="sm")
                        nc.tensor.matmul(ss_ps[0:HPC, :], hsum[:], sq[:],
                                         start=True, stop=True)
                        rnsq = spool.tile([HPC, 512], F32, tag="rnsq")
                        nc.scalar.activation(rnsq[:], ss_ps[0:HPC, :], AF.Sqrt,
                                             bias=epskq_t[0:HPC, :])
                        rn = spool.tile([HPC, 512], F32, tag="rn")
                        nc.vector.reciprocal_approx_fast(rn[:], rnsq[:])
                        rnb = spool.tile([HPC, 512], BF16, tag="rnb")
                        nc.vector.tensor_copy(rnb[:], rn[:])
                        bc_ps = ps_sm.tile([128, 512], F32, tag="sm")
                        nc.tensor.matmul(bc_ps[:], hsel[:], rnb[:],
                                         start=True, stop=True)
                        nc.vector.tensor_mul(raw[:, csl], raw[:, csl],
                                             bc_ps[:])

                l2norm(qT)
                l2norm(kT)

                # ---- v scaled by r1 in place (feature-major) ----
                mvv = spool.tile([128, 1], F32, tag="mvv")
                mv4 = spool.tile([128, NQC], F32, tag="mv4")
                for ch in range(NQC):
                    csl = slice(ch * 512, (ch + 1) * 512)
                    bc = rpool.tile([128, 512], F32, tag="bc")
                    nc.gpsimd.partition_broadcast(bc[:], r1[:, csl],
                                                  channels=128)
                    nc.vector.tensor_mul(vT_s[:, csl], vT_s[:, csl], bc[:])
                    nc.vector.reduce_sum(mv4[:, ch:ch + 1], vT_s[:, csl],
                                         mybir.AxisListType.X)
                nc.vector.reduce_sum(mvv[:], mv4[:], mybir.AxisListType.X)

                v_augs = [[], []]
                for kt in range(S // 128):
                    for h in range(HPC):
                        va = vpool.tile([128, D + 1], BF16, tag=f"va{h}_{kt}")
                        nc.vector.tensor_copy(va[:, D:D + 1], ones128[:])
                        tr_ps = ps_sm.tile([128, 64], BF16, tag="sm")
                        nc.tensor.transpose(
                            tr_ps[0:128, 0:64],
                            vT_s[h * D:(h + 1) * D, kt * 128:(kt + 1) * 128],
                            ident[h * D:(h + 1) * D, :])
                        nc.vector.tensor_copy(va[:, 0:D], tr_ps[0:128, 0:64])
                        v_augs[h].append(va)

                # ---- attention ----
                # exp over paired k-tiles: p = exp(s*scale_h + fconst_h),
                # near-band tiles then multiplied by exp(bias - fconst).
                for h in range(HPC):
                    hd = slice(h * D, (h + 1) * D)
                    for qc in range(NQC):
                        Q0 = qc * 512
                        qsl = slice(Q0, Q0 + 512)
                        nkt = (Q0 + 512) // 128
                        o_ps = ps_acc.tile([128, 512], F32, tag="acc")
                        npair = nkt // 2

                        def emit_pv(pr, p_t):
                            for t2 in range(2):
                                kt = 2 * pr + t2
                                psl = slice(t2 * 512, (t2 + 1) * 512)
                                nc.tensor.matmul(
                                    o_ps[0:D + 1, :], v_augs[h][kt][:],
                                    p_t[:, psl],
                                    start=(kt == 0), stop=(kt == nkt - 1))

                        pending = []
                        for pr in range(npair):
                            pp = ps_pair.tile([128, 1024], F32, tag="pair")
                            for t2 in range(2):
                                K0 = (2 * pr + t2) * 128
                                nc.tensor.matmul(
                                    pp[:, t2 * 512:(t2 + 1) * 512],
                                    kT[hd, K0:K0 + 128], qT[hd, qsl],
                                    start=True, stop=True)
                            p_t = ppool.tile([128, 1024], BF16, tag="p")
                            nc.scalar.activation(
                                p_t[:], pp[:], AF.Exp,
                                bias=fconst_t[:, h:h + 1],
                                scale=scaleb_t[:, h:h + 1])
                            for t2 in range(2):
                                kt = 2 * pr + t2
                                psl = slice(t2 * 512, (t2 + 1) * 512)
                                di = 1 + kt - 4 * qc
                                if 0 <= di <= 4:
                                    bsl = slice((h * 5 + di) * 512,
                                                (h * 5 + di + 1) * 512)
                                    nc.vector.tensor_mul(
                                        p_t[:, psl], p_t[:, psl],
                                        expb[:, bsl])
                            pending.append((pr, p_t))
                            # keep the PE one score-pair ahead of the PVs
                            if len(pending) >= 2:
                                emit_pv(*pending.pop(0))
                        for item in pending:
                            emit_pv(*item)
                        den = spool.tile([1, 512], F32, tag="den")
                        nc.vector.tensor_copy(den[:], o_ps[D:D + 1, :])
                        srec = spool.tile([1, 512], F32, tag="srec")
                        nc.vector.reciprocal_approx_fast(srec[:], den[:])
                        nb = rpool.tile([64, 512], F32, tag="nb")
                        nc.gpsimd.partition_broadcast(nb[:], srec[:],
                                                      channels=D)
                        o_n = opool.tile([64, 512], BF16, tag="on")
                        nc.vector.tensor_mul(o_n[:], o_ps[0:D, :], nb[:])
                        if qc == 0:
                            nc.vector.tensor_scalar_mul(
                                o_n[:, 0:1], mvv[h * D:(h + 1) * D, :],
                                1.0 / S)
                        nc.sync.dma_start(
                            oT_d[hd, b * S + Q0: b * S + Q0 + 512], o_n[:])
    nc.compile()
    return nc


def prep_a_inputs(inputs):
    x = np.asarray(inputs["x"], np.float32)
    ln1 = np.asarray(inputs["ln1_w"], np.float32)
    Wq = np.asarray(inputs["Wq"], np.float32)
    Wk = np.asarray(inputs["Wk"], np.float32)
    Wv = np.asarray(inputs["Wv"], np.float32)
    rb = np.asarray(inputs["rel_bias"], np.float32)
    scale = np.asarray(inputs["scale"], np.float32)
    xT = np.ascontiguousarray(x.reshape(B * S, E).T).astype(NP_BF16)
    d = np.arange(2048) - BOFF
    bucket = rel_bucket(np.maximum(d, 1))
    biasv_all = np.where(
        (d < 1)[None, :], np.float32(MASKVAL),
        scale[:, None] * rb[bucket, :].T.astype(np.float32)).astype(np.float32)
    fconst_all = (scale * rb[NB - 1, :]).astype(np.float32)
    # exp(bias - fconst) delta-tiles: [H, 5, 128, 512];
    # E[h,di,p,f] = exp(v_h[BOFF+128-128*di+f-p] - fconst_h)
    expv_all = np.exp(biasv_all - fconst_all[:, None]).astype(np.float32)
    di_ = np.arange(5)[:, None, None]
    p_ = np.arange(128)[None, :, None]
    f_ = np.arange(512)[None, None, :]
    idx = BOFF + 128 - 128 * di_ + f_ - p_
    expb_all = expv_all[:, idx].astype(NP_BF16)  # [H, 5, 128, 512]

    def tile_w(w):  # [1024, M] -> [128, 8*M]
        M = w.shape[1]
        return np.ascontiguousarray(
            w.reshape(8, 128, M).transpose(1, 0, 2).reshape(128, 8 * M))
    ident_np = np.concatenate([np.eye(D, dtype=np.float32)] * 2,
                              axis=0).astype(NP_BF16)
    in_maps = []
    for c in range(NCORE):
        hs = slice(c * HPC, (c + 1) * HPC)
        cs = slice(c * HPC * D, (c + 1) * HPC * D)
        hsum_np = np.zeros((128, HPC), np.float32)
        for h in range(HPC):
            hsum_np[h * D:(h + 1) * D, h] = 1.0
        hsel_np = np.zeros((HPC, 128), np.float32)
        for h in range(HPC):
            hsel_np[h, h * D:(h + 1) * D] = 1.0
        in_maps.append({
            "xT": xT,
            "hsel": hsel_np.astype(NP_BF16),
            "identc": ident_np,
            "onesc": np.ones((128, 1), NP_BF16),
            "hsum": hsum_np.astype(NP_BF16),
            "wq": tile_w(ln1[:, None] * Wq[:, cs]).astype(NP_BF16),
            "wk": tile_w(ln1[:, None] * Wk[:, cs]).astype(NP_BF16),
            "wv": tile_w(ln1[:, None] * Wv[:, cs]).astype(NP_BF16),
            "expb": np.ascontiguousarray(
                expb_all[hs].reshape(HPC * 5, 128, 512)),
            "fconst": np.ascontiguousarray(
                np.broadcast_to(fconst_all[hs], (128, HPC))),
            "scaleb": np.ascontiguousarray(
                np.broadcast_to(scale[hs], (128, HPC))),
        })
    return in_maps


def build_launch_b():
    nc = bacc.Bacc("TRN2", target_bir_lowering=False, debug=False)
    oT_d = nc.dram_tensor("oTs", [E, TPC], BF16, kind="ExternalInput").ap()
    xT_d = nc.dram_tensor("xTs", [E, TPC], F32R, kind="ExternalInput").ap()
    wo_d = nc.dram_tensor("wo", [E // 128, 128, E], BF16, kind="ExternalInput")
    w1_d = nc.dram_tensor("w1", [HID // 128, 128, E], BF16, kind="ExternalInput")
    w2_d = nc.dram_tensor("w2", [E // 128, 128, HID], BF16, kind="ExternalInput")
    onesc_d = nc.dram_tensor("onesc", [128, 1], BF16, kind="ExternalInput").ap()
    onesr_d = nc.dram_tensor("onesr", [1, 128], F32, kind="ExternalInput").ap()
    out_d = nc.dram_tensor("outT", [E, TPC], F32, kind="ExternalOutput").ap()

    NE = E // 128    # 8 e-tiles
    NH = HID // 128  # 32 h-tiles

    with tile.TileContext(nc) as tc:
        with nc.allow_low_precision(reason="bf16 kernel"), \
             tc.tile_pool(name="const", bufs=1) as cpool, \
             tc.tile_pool(name="io", bufs=1) as iopool, \
             tc.tile_pool(name="y", bufs=1) as ypool, \
             tc.tile_pool(name="h", bufs=1) as hpool, \
             tc.tile_pool(name="w1s", bufs=3) as w1pool, \
             tc.tile_pool(name="w2s", bufs=3) as w2pool, \
             tc.tile_pool(name="small", bufs=2) as spool, \
             tc.tile_pool(name="ps_mm", bufs=4, space="PSUM") as ps_mm, \
             tc.tile_pool(name="ps_acc", bufs=2, space="PSUM") as ps_acc:

            ones128 = cpool.tile([128, 1], BF16)
            nc.sync.dma_start(ones128[:], onesc_d[:])
            epsln_t = cpool.tile([128, 1], F32)
            nc.vector.memset(epsln_t[:], LN_EPS)

            oTs, xTs = [], []
            for i in range(NE):
                ot = iopool.tile([128, TPC], BF16, tag=f"o{i}")
                nc.sync.dma_start(ot[:], oT_d[i * 128:(i + 1) * 128, :])
                oTs.append(ot)
                xt = iopool.tile([128, TPC], F32R, tag=f"xs{i}")
                nc.sync.dma_start(xt[:], xT_d[i * 128:(i + 1) * 128, :])
                xTs.append(xt)
            # ---- y^T = Wo^T @ O^T + x^T ----
            yTs, yBs = [], []
            for i in range(NE):
                wo_t = w1pool.tile([128, NE, 128], BF16, tag="w1")
                nc.sync.dma_start(
                    wo_t[:], bass_rust.AP(wo_d, i * 128 * E,
                                          [[E, 128], [1, NE * 128]]))
                ps = ps_acc.tile([128, TPC], F32, tag="acc")
                for j in range(NE):
                    nc.tensor.matmul(ps[:], wo_t[:, j, :],
                                     oTs[j][:], start=(j == 0), stop=(j == NE - 1))
                yt = ypool.tile([128, TPC], F32R, tag=f"y{i}")
                nc.vector.tensor_add(yt[:], ps[:], xTs[i][:])
                yTs.append(yt)
                yb = ypool.tile([128, TPC], BF16, tag=f"yb{i}")
                nc.scalar.activation(yb[:], yt[:], AF.Copy)
                yBs.append(yb)

            # ---- r2 = 1/sqrt(mean(y^2)+eps) ----
            ssy_ps = ps_acc.tile([128, TPC], F32, tag="acc")
            for i in range(NE):
                sq = spool.tile([128, TPC], BF16, tag="sqy")
                nc.vector.tensor_mul(sq[:], yBs[i][:], yBs[i][:])
                nc.tensor.matmul(ssy_ps[0:1, :], ones128[:], sq[:],
                                 start=(i == 0), stop=(i == NE - 1))
            r2sq = spool.tile([1, TPC], F32, tag="r2sq")
            nc.scalar.activation(r2sq[:], ssy_ps[0:1, :], AF.Sqrt,
                                 bias=epsln_t[0:1, :], scale=1.0 / E)
            r2 = spool.tile([1, TPC], F32, tag="r2")
            nc.vector.reciprocal_approx_fast(r2[:], r2sq[:])
            # broadcast r2 over 128 partitions
            r2b_ps = ps_mm.tile([128, TPC], F32, tag="mm")
            o1x128 = cpool.tile([1, 128], F32)
            nc.sync.dma_start(o1x128[:], onesr_d[:])
            nc.tensor.matmul(r2b_ps[:], o1x128[:], r2[:], start=True, stop=True)
            r2b = cpool.tile([128, TPC], F32)
            nc.scalar.activation(r2b[:], r2b_ps[:], AF.Copy)

            # ---- h^T = relu(W1'^T y^T) ----
            hts = []
            for ht in range(NH):
                w1_t = w1pool.tile([128, NE, 128], BF16, tag="w1")
                nc.sync.dma_start(
                    w1_t[:], bass_rust.AP(w1_d, ht * 128 * E,
                                          [[E, 128], [1, NE * 128]]))
                ps = ps_mm.tile([128, TPC], F32, tag="mm")
                for j in range(NE):
                    nc.tensor.matmul(ps[:], w1_t[:, j, :], yBs[j][:],
                                     start=(j == 0), stop=(j == NE - 1))
                h_t = hpool.tile([128, TPC], BF16, tag=f"h{ht}")
                nc.scalar.activation(h_t[:], ps[:], AF.Relu)
                hts.append(h_t)

            # ---- z = (h^T' W2)^T * r2 + y ----
            for i in range(NE):
                w2_t = w2pool.tile([128, NH, 128], BF16, tag="w2")
                nc.sync.dma_start(
                    w2_t[:], bass_rust.AP(w2_d, i * 128 * HID,
                                          [[HID, 128], [1, NH * 128]]))
                ps = ps_acc.tile([128, TPC], F32, tag="acc")
                for ht in range(NH):
                    nc.tensor.matmul(ps[:], w2_t[:, ht, :], hts[ht][:],
                                     start=(ht == 0), stop=(ht == NH - 1))
                zt = spool.tile([128, TPC], F32, tag="zt")
                nc.vector.tensor_mul(zt[:], ps[:], r2b[:])
                outt = spool.tile([128, TPC], F32, tag="outt")
                nc.vector.tensor_add(outt[:], zt[:], yTs[i][:])
                nc.sync.dma_start(out_d[i * 128:(i + 1) * 128, :], outt[:])
    nc.compile()
    return nc


def prep_b_inputs(inputs, oT_all):
    x = np.asarray(inputs["x"], np.float32)
    ln2 = np.asarray(inputs["ln2_w"], np.float32)
    def tile_cols(w):
        # [K, M] -> [M//128, 128, K]: out[i, p, j*128+m] = w[j*128+p, i*128+m]
        K, M = w.shape
        return np.ascontiguousarray(
            w.reshape(K // 128, 128, M // 128, 128)
            .transpose(2, 1, 0, 3).reshape(M // 128, 128, K))
    Wo = tile_cols(np.asarray(inputs["Wo"], np.float32)).astype(NP_BF16)
    W1 = tile_cols(
        ln2[:, None] * np.asarray(inputs["W1"], np.float32)).astype(NP_BF16)
    W2 = tile_cols(np.asarray(inputs["W2"], np.float32)).astype(NP_BF16)
    xT = x.reshape(B * S, E).T
    in_maps = []
    for c in range(NCORE):
        ts = slice(c * TPC, (c + 1) * TPC)
        in_maps.append({
            "oTs": np.ascontiguousarray(oT_all[:, ts]).astype(NP_BF16),
            "onesc": np.ones((128, 1), NP_BF16),
            "onesr": np.ones((1, 128), np.float32),
            "xTs": np.ascontiguousarray(xT[:, ts]),
            "wo": Wo, "w1": W1, "w2": W2,
        })
    return in_maps


_CACHE = {}


def _get_compiled():
    if "a" not in _CACHE:
        _CACHE["a"] = build_launch_a()
    if "b" not in _CACHE:
        _CACHE["b"] = build_launch_b()
    return _CACHE["a"], _CACHE["b"]


def kernel(**inputs):
    from concourse import bass_utils
    inputs = {k: np.asarray(v) for k, v in inputs.items()}
    nca, ncb = _get_compiled()
    in_maps_a = prep_a_inputs(inputs)
    res_a = bass_utils.run_bass_kernel_spmd(
        nca, in_maps_a, core_ids=list(range(NCORE)))
    oT_all = np.concatenate([res_a.results[c]["oT"] for c in range(NCORE)],
                            axis=0)  # [E, B*S], rows = h*64+d
    in_maps_b = prep_b_inputs(inputs, oT_all)
    res_b = bass_utils.run_bass_kernel_spmd(
        ncb, in_maps_b, core_ids=list(range(NCORE)))
    outT = np.concatenate([res_b.results[c]["outT"] for c in range(NCORE)],
                          axis=1)  # [E, B*S]
    return np.ascontiguousarray(outT.T).reshape(B, S, E).astype(np.float32)



# revision 24
# speedup vs baseline: 1.7695x; 1.0021x over previous
# Trainium2 Bass kernels for nn_DecoderLayer (T5-style decoder layer).
# Two SPMD launches over 8 cores:
#   A: head-parallel attention (2 heads/core) -> per-head attn outputs O^T
#   B: token-parallel output-proj + FFN (512 tokens/core)
# Activations kept feature-major (features on partitions).
import sys
sys.path.insert(0, '/opt/trn_rl_repo')
import math
import numpy as np
import bass_rust
import concourse.bass as bass
import concourse.mybir as mybir
import concourse.tile as tile
from concourse import bacc

F32 = mybir.dt.float32
F32R = mybir.dt.float32r
BF16 = mybir.dt.bfloat16
FP8 = mybir.dt.float8e4
AF = mybir.ActivationFunctionType
DR = mybir.MatmulPerfMode.DoubleRow

import ml_dtypes
NP_BF16 = ml_dtypes.bfloat16
NP_FP8 = ml_dtypes.float8_e4m3fn

E = 1024
H = 16
D = 64
HID = 4096
B = 2
S = 2048
NB = 32
MAXD = 128
LN_EPS = 1e-6
KQ_EPS = 1e-6
NCORE = 8
HPC = H // NCORE          # heads per core (2)
TPC = B * S // NCORE      # tokens per core (512)
NQC = S // 512            # q-chunks per batch (4)
BOFF = 1024               # bias vec offset: index j = d + BOFF
MASKVAL = -20.0


def rel_bucket(d):
    d = np.asarray(d)
    max_exact = NB // 2
    safe = np.maximum(d, 1).astype(np.float64)
    large = max_exact + (
        np.log(safe / max_exact) / math.log(MAXD / max_exact) * (NB - max_exact)
    ).astype(np.int32)
    large = np.minimum(large, NB - 1)
    return np.where(d < max_exact, d, large)


def build_launch_a():
    nc = bacc.Bacc("TRN2", target_bir_lowering=False, debug=False)
    xT_d = nc.dram_tensor("xT", [E, B * S], BF16, kind="ExternalInput").ap()
    wq_d = nc.dram_tensor("wq", [128, 8 * HPC * D], BF16, kind="ExternalInput").ap()
    wk_d = nc.dram_tensor("wk", [128, 8 * HPC * D], BF16, kind="ExternalInput").ap()
    wv_d = nc.dram_tensor("wv", [128, 8 * HPC * D], BF16, kind="ExternalInput").ap()
    expb_d = nc.dram_tensor("expb", [HPC * 5, 128, 512], BF16,
                            kind="ExternalInput")
    fconst_d = nc.dram_tensor("fconst", [128, HPC], F32, kind="ExternalInput").ap()
    scaleb_d = nc.dram_tensor("scaleb", [128, HPC], F32, kind="ExternalInput").ap()
    onesc_d = nc.dram_tensor("onesc", [128, 1], BF16, kind="ExternalInput").ap()
    hsum_d = nc.dram_tensor("hsum", [128, HPC], BF16, kind="ExternalInput").ap()
    hsel_d = nc.dram_tensor("hsel", [HPC, 128], BF16, kind="ExternalInput").ap()
    identc_d = nc.dram_tensor("identc", [128, 64], BF16, kind="ExternalInput").ap()
    oT_d = nc.dram_tensor("oT", [HPC * D, B * S], BF16, kind="ExternalOutput").ap()

    with tile.TileContext(nc) as tc:
        with nc.allow_low_precision(reason="bf16 kernel"), \
             tc.tile_pool(name="const", bufs=1) as cpool, \
             tc.tile_pool(name="xt", bufs=2) as xpool, \
             tc.tile_pool(name="w", bufs=1) as wpool, \
             tc.tile_pool(name="qkv", bufs=2) as qkvpool, \
             tc.tile_pool(name="vtok", bufs=2) as vpool, \
             tc.tile_pool(name="ptile", bufs=4) as ppool, \
             tc.tile_pool(name="onorm", bufs=3) as opool, \
             tc.tile_pool(name="small", bufs=3) as spool, \
             tc.tile_pool(name="r1p", bufs=2) as rpool, \
             tc.tile_pool(name="ps_pair", bufs=2, space="PSUM") as ps_pair, \
             tc.tile_pool(name="ps_sm", bufs=2, space="PSUM") as ps_sm, \
             tc.tile_pool(name="ps_acc", bufs=2, space="PSUM") as ps_acc:

            # ---- constants ----
            fconst_t = cpool.tile([128, HPC], F32)
            nc.sync.dma_start(fconst_t[:], fconst_d[:])
            scaleb_t = cpool.tile([128, HPC], F32)
            nc.sync.dma_start(scaleb_t[:], scaleb_d[:])
            ones128 = cpool.tile([128, 1], BF16)
            nc.sync.dma_start(ones128[:], onesc_d[:])
            hsum = cpool.tile([128, HPC], BF16)
            nc.sync.dma_start(hsum[:], hsum_d[:])
            hsel = cpool.tile([HPC, 128], BF16)
            nc.sync.dma_start(hsel[:], hsel_d[:])
            epsln_t = cpool.tile([128, 1], F32)
            nc.vector.memset(epsln_t[:], LN_EPS)
            epskq_t = cpool.tile([128, 1], F32)
            nc.vector.memset(epskq_t[:], KQ_EPS)
            # stacked identities [128, 64] for per-head PE transpose
            ident = cpool.tile([128, 64], BF16)
            nc.sync.dma_start(ident[:], identc_d[:])
            # near-band exp(bias - fconst) tiles: [128, (h*5+di)*512 + f]
            expb = cpool.tile([128, HPC * 5 * 512], BF16)
            nc.sync.dma_start(
                expb[:],
                bass_rust.AP(expb_d, 0, [[512, 128], [65536, HPC * 5],
                                         [1, 512]]))

            # weights (ln1-folded, bf16): [128, e-tile, col]
            wq_t = wpool.tile([128, 8, HPC * D], BF16, tag="wq")
            wk_t = wpool.tile([128, 8, HPC * D], BF16, tag="wk")
            wv_t = wpool.tile([128, 8, HPC * D], BF16, tag="wv")
            for (w_t, w_d) in ((wq_t, wq_d), (wk_t, wk_d), (wv_t, wv_d)):
                nc.sync.dma_start(
                    w_t[:], w_d.rearrange("p (i m) -> p i m", i=8))

            for b in range(B):
                # ---- xT e-tiles [128, 2048] ----
                xts = []
                for i in range(8):
                    xt = xpool.tile([128, S], BF16, tag=f"x{i}")
                    nc.sync.dma_start(xt[:], xT_d[i * 128:(i + 1) * 128,
                                                  b * S:(b + 1) * S])
                    xts.append(xt)

                # ---- r1 = 1/sqrt(mean(x^2)+eps) ----
                r1 = rpool.tile([1, S], F32, tag="r1")
                for ch in range(S // 512):
                    csl = slice(ch * 512, (ch + 1) * 512)
                    # per-etile squares, tree-added on DVE; single matmul
                    # reduces the 128 partitions.
                    ssq = spool.tile([128, 512], BF16, tag="ssq")
                    nc.vector.tensor_mul(ssq[:], xts[0][:, csl],
                                         xts[0][:, csl])
                    for i in range(1, 8):
                        sq = spool.tile([128, 512], BF16, tag="sqx")
                        nc.vector.tensor_mul(sq[:], xts[i][:, csl],
                                             xts[i][:, csl])
                        nc.vector.tensor_add(ssq[:], ssq[:], sq[:])
                    ssx_ps = ps_sm.tile([128, 512], F32, tag="sm")
                    nc.tensor.matmul(ssx_ps[0:1, :], ones128[:], ssq[:],
                                     start=True, stop=True)
                    r1sq = spool.tile([1, 512], F32, tag="r1sq")
                    nc.scalar.activation(r1sq[:], ssx_ps[0:1, :], AF.Sqrt,
                                         bias=epsln_t[0:1, :], scale=1.0 / E)
                    nc.vector.reciprocal_approx_fast(r1[:, csl], r1sq[:])

                # ---- projections: raw qT/kT/vT packed [128, 2048] ----
                # i-outer loop reuses the stationary weight tile across the
                # 4 token chunks (2 psum pair-tiles of 2 chunks each).
                def project(w_t, name, copy_eng):
                    out = qkvpool.tile([128, S], BF16, tag=name)
                    pp0 = ps_pair.tile([128, 1024], F32, tag="pair")
                    pp1 = ps_pair.tile([128, 1024], F32, tag="pair")
                    pps = (pp0, pp1)
                    for i in range(8):
                        for ch in range(4):
                            csl = slice(ch * 512, (ch + 1) * 512)
                            pp = pps[ch // 2]
                            off = (ch % 2) * 512
                            nc.tensor.matmul(
                                pp[:, off:off + 512], w_t[:, i, :],
                                xts[i][:, csl], start=(i == 0), stop=(i == 7))
                    for j in range(2):
                        osl = slice(j * 1024, (j + 1) * 1024)
                        if copy_eng == "scalar":
                            nc.scalar.activation(out[:, osl], pps[j][:],
                                                 AF.Copy)
                        else:
                            nc.vector.tensor_copy(out[:, osl], pps[j][:])
                    return out

                qT = project(wq_t, "qT", "scalar")
                kT = project(wk_t, "kT", "vector")
                vT_s = project(wv_t, "vT", "vector")

                # ---- L2-normalize q and k in place ----
                # fp8 normalized q/k: [128, 2*S]; cols S..2S are zeros so a
                # [64, 2, n] slice pairs real data with a zero contraction
                # block for DoubleRow matmuls.
                qT8 = qkvpool.tile([128, 2 * S], FP8, tag="qT8")
                kT8 = qkvpool.tile([128, 2 * S], FP8, tag="kT8")
                nc.vector.memset(qT8[:, S:], 0.0)
                nc.vector.memset(kT8[:, S:], 0.0)

                def l2norm(raw, out8):
                    for ch in range(S // 512):
                        csl = slice(ch * 512, (ch + 1) * 512)
                        sq = spool.tile([128, 512], BF16, tag="sqn")
                        nc.vector.tensor_mul(sq[:], raw[:, csl], raw[:, csl])
                        ss_ps = ps_sm.tile([128, 512], F32, tag="sm")
                        nc.tensor.matmul(ss_ps[0:HPC, :], hsum[:], sq[:],
                                         start=True, stop=True)
                        rnsq = spool.tile([HPC, 512], F32, tag="rnsq")
                        nc.scalar.activation(rnsq[:], ss_ps[0:HPC, :], AF.Sqrt,
                                             bias=epskq_t[0:HPC, :])
                        rn = spool.tile([HPC, 512], F32, tag="rn")
                        nc.vector.reciprocal_approx_fast(rn[:], rnsq[:])
                        rnb = spool.tile([HPC, 512], BF16, tag="rnb")
                        nc.vector.tensor_copy(rnb[:], rn[:])
                        bc_ps = ps_sm.tile([128, 512], F32, tag="sm")
                        nc.tensor.matmul(bc_ps[:], hsel[:], rnb[:],
                                         start=True, stop=True)
                        nc.vector.tensor_mul(out8[:, csl], raw[:, csl],
                                             bc_ps[:])

                l2norm(qT, qT8)
                l2norm(kT, kT8)

                # ---- v scaled by r1 in place (feature-major) ----
                mvv = spool.tile([128, 1], F32, tag="mvv")
                mv4 = spool.tile([128, NQC], F32, tag="mv4")
                for ch in range(NQC):
                    csl = slice(ch * 512, (ch + 1) * 512)
                    bc = rpool.tile([128, 512], F32, tag="bc")
                    nc.gpsimd.partition_broadcast(bc[:], r1[:, csl],
                                                  channels=128)
                    nc.vector.tensor_mul(vT_s[:, csl], vT_s[:, csl], bc[:])
                    nc.vector.reduce_sum(mv4[:, ch:ch + 1], vT_s[:, csl],
                                         mybir.AxisListType.X)
                nc.vector.reduce_sum(mvv[:], mv4[:], mybir.AxisListType.X)

                v_augs = [[], []]
                for kt in range(S // 128):
                    for h in range(HPC):
                        va = vpool.tile([128, D + 1], BF16, tag=f"va{h}_{kt}")
                        nc.vector.tensor_copy(va[:, D:D + 1], ones128[:])
                        tr_ps = ps_sm.tile([128, 64], BF16, tag="sm")
                        nc.tensor.transpose(
                            tr_ps[0:128, 0:64],
                            vT_s[h * D:(h + 1) * D, kt * 128:(kt + 1) * 128],
                            ident[h * D:(h + 1) * D, :])
                        nc.vector.tensor_copy(va[:, 0:D], tr_ps[0:128, 0:64])
                        v_augs[h].append(va)

                # ---- attention ----
                # exp over paired k-tiles: p = exp(s*scale_h + fconst_h),
                # near-band tiles then multiplied by exp(bias - fconst).
                for h in range(HPC):
                    hd = slice(h * D, (h + 1) * D)
                    k8h = kT8[hd, :].rearrange("p (j c) -> p j c", j=2)
                    q8h = qT8[hd, :].rearrange("p (j c) -> p j c", j=2)
                    for qc in range(NQC):
                        Q0 = qc * 512
                        qsl = slice(Q0, Q0 + 512)
                        nkt = (Q0 + 512) // 128
                        o_ps = ps_acc.tile([128, 512], F32, tag="acc")
                        npair = nkt // 2

                        def emit_pv(pr, p_t):
                            for t2 in range(2):
                                kt = 2 * pr + t2
                                psl = slice(t2 * 512, (t2 + 1) * 512)
                                nc.tensor.matmul(
                                    o_ps[0:D + 1, :], v_augs[h][kt][:],
                                    p_t[:, psl],
                                    start=(kt == 0), stop=(kt == nkt - 1))

                        pending = []
                        for pr in range(npair):
                            pp = ps_pair.tile([128, 1024], F32, tag="pair")
                            for t2 in range(2):
                                K0 = (2 * pr + t2) * 128
                                nc.tensor.matmul(
                                    pp[:, t2 * 512:(t2 + 1) * 512],
                                    k8h[:, :, K0:K0 + 128],
                                    q8h[:, :, qsl],
                                    start=True, stop=True, perf_mode=DR)
                            p_t = ppool.tile([128, 1024], BF16, tag="p")
                            nc.scalar.activation(
                                p_t[:], pp[:], AF.Exp,
                                bias=fconst_t[:, h:h + 1],
                                scale=scaleb_t[:, h:h + 1])
                            for t2 in range(2):
                                kt = 2 * pr + t2
                                psl = slice(t2 * 512, (t2 + 1) * 512)
                                di = 1 + kt - 4 * qc
                                if 0 <= di <= 4:
                                    bsl = slice((h * 5 + di) * 512,
                                                (h * 5 + di + 1) * 512)
                                    nc.vector.tensor_mul(
                                        p_t[:, psl], p_t[:, psl],
                                        expb[:, bsl])
                            pending.append((pr, p_t))
                            # keep the PE one score-pair ahead of the PVs
                            if len(pending) >= 2:
                                emit_pv(*pending.pop(0))
                        for item in pending:
                            emit_pv(*item)
                        den = spool.tile([1, 512], F32, tag="den")
                        nc.vector.tensor_copy(den[:], o_ps[D:D + 1, :])
                        srec = spool.tile([1, 512], F32, tag="srec")
                        nc.vector.reciprocal_approx_fast(srec[:], den[:])
                        nb = rpool.tile([64, 512], F32, tag="nb")
                        nc.gpsimd.partition_broadcast(nb[:], srec[:],
                                                      channels=D)
                        o_n = opool.tile([64, 512], BF16, tag="on")
                        nc.vector.tensor_mul(o_n[:], o_ps[0:D, :], nb[:])
                        if qc == 0:
                            nc.vector.tensor_scalar_mul(
                                o_n[:, 0:1], mvv[h * D:(h + 1) * D, :],
                                1.0 / S)
                        nc.sync.dma_start(
                            oT_d[hd, b * S + Q0: b * S + Q0 + 512], o_n[:])
    nc.compile()
    return nc


def prep_a_inputs(inputs):
    x = np.asarray(inputs["x"], np.float32)
    ln1 = np.asarray(inputs["ln1_w"], np.float32)
    Wq = np.asarray(inputs["Wq"], np.float32)
    Wk = np.asarray(inputs["Wk"], np.float32)
    Wv = np.asarray(inputs["Wv"], np.float32)
    rb = np.asarray(inputs["rel_bias"], np.float32)
    scale = np.asarray(inputs["scale"], np.float32)
    xT = np.ascontiguousarray(x.reshape(B * S, E).T).astype(NP_BF16)
    d = np.arange(2048) - BOFF
    bucket = rel_bucket(np.maximum(d, 1))
    biasv_all = np.where(
        (d < 1)[None, :], np.float32(MASKVAL),
        scale[:, None] * rb[bucket, :].T.astype(np.float32)).astype(np.float32)
    fconst_all = (scale * rb[NB - 1, :]).astype(np.float32)
    # exp(bias - fconst) delta-tiles: [H, 5, 128, 512];
    # E[h,di,p,f] = exp(v_h[BOFF+128-128*di+f-p] - fconst_h)
    expv_all = np.exp(biasv_all - fconst_all[:, None]).astype(np.float32)
    di_ = np.arange(5)[:, None, None]
    p_ = np.arange(128)[None, :, None]
    f_ = np.arange(512)[None, None, :]
    idx = BOFF + 128 - 128 * di_ + f_ - p_
    expb_all = expv_all[:, idx].astype(NP_BF16)  # [H, 5, 128, 512]

    def tile_w(w):  # [1024, M] -> [128, 8*M]
        M = w.shape[1]
        return np.ascontiguousarray(
            w.reshape(8, 128, M).transpose(1, 0, 2).reshape(128, 8 * M))
    ident_np = np.concatenate([np.eye(D, dtype=np.float32)] * 2,
                              axis=0).astype(NP_BF16)
    in_maps = []
    for c in range(NCORE):
        hs = slice(c * HPC, (c + 1) * HPC)
        cs = slice(c * HPC * D, (c + 1) * HPC * D)
        hsum_np = np.zeros((128, HPC), np.float32)
        for h in range(HPC):
            hsum_np[h * D:(h + 1) * D, h] = 1.0
        hsel_np = np.zeros((HPC, 128), np.float32)
        for h in range(HPC):
            hsel_np[h, h * D:(h + 1) * D] = 1.0
        in_maps.append({
            "xT": xT,
            "hsel": hsel_np.astype(NP_BF16),
            "identc": ident_np,
            "onesc": np.ones((128, 1), NP_BF16),
            "hsum": hsum_np.astype(NP_BF16),
            "wq": tile_w(ln1[:, None] * Wq[:, cs]).astype(NP_BF16),
            "wk": tile_w(ln1[:, None] * Wk[:, cs]).astype(NP_BF16),
            "wv": tile_w(ln1[:, None] * Wv[:, cs]).astype(NP_BF16),
            "expb": np.ascontiguousarray(
                expb_all[hs].reshape(HPC * 5, 128, 512)),
            "fconst": np.ascontiguousarray(
                np.broadcast_to(fconst_all[hs], (128, HPC))),
            "scaleb": np.ascontiguousarray(
                np.broadcast_to(scale[hs], (128, HPC))),
        })
    return in_maps


def build_launch_b():
    nc = bacc.Bacc("TRN2", target_bir_lowering=False, debug=False)
    oT_d = nc.dram_tensor("oTs", [E, TPC], BF16, kind="ExternalInput").ap()
    xT_d = nc.dram_tensor("xTs", [E, TPC], F32R, kind="ExternalInput").ap()
    wo_d = nc.dram_tensor("wo", [E // 128, 128, E], BF16, kind="ExternalInput")
    w1_d = nc.dram_tensor("w1", [HID // 128, 128, E], BF16, kind="ExternalInput")
    w2_d = nc.dram_tensor("w2", [E // 128, 128, HID], BF16, kind="ExternalInput")
    onesc_d = nc.dram_tensor("onesc", [128, 1], BF16, kind="ExternalInput").ap()
    onesr_d = nc.dram_tensor("onesr", [1, 128], F32, kind="ExternalInput").ap()
    out_d = nc.dram_tensor("outT", [E, TPC], F32, kind="ExternalOutput").ap()

    NE = E // 128    # 8 e-tiles
    NH = HID // 128  # 32 h-tiles

    with tile.TileContext(nc) as tc:
        with nc.allow_low_precision(reason="bf16 kernel"), \
             tc.tile_pool(name="const", bufs=1) as cpool, \
             tc.tile_pool(name="io", bufs=1) as iopool, \
             tc.tile_pool(name="y", bufs=1) as ypool, \
             tc.tile_pool(name="h", bufs=1) as hpool, \
             tc.tile_pool(name="w1s", bufs=3) as w1pool, \
             tc.tile_pool(name="w2s", bufs=3) as w2pool, \
             tc.tile_pool(name="small", bufs=2) as spool, \
             tc.tile_pool(name="ps_mm", bufs=4, space="PSUM") as ps_mm, \
             tc.tile_pool(name="ps_acc", bufs=2, space="PSUM") as ps_acc:

            ones128 = cpool.tile([128, 1], BF16)
            nc.sync.dma_start(ones128[:], onesc_d[:])
            epsln_t = cpool.tile([128, 1], F32)
            nc.vector.memset(epsln_t[:], LN_EPS)

            oTs, xTs = [], []
            for i in range(NE):
                ot = iopool.tile([128, TPC], BF16, tag=f"o{i}")
                nc.sync.dma_start(ot[:], oT_d[i * 128:(i + 1) * 128, :])
                oTs.append(ot)
                xt = iopool.tile([128, TPC], F32R, tag=f"xs{i}")
                nc.sync.dma_start(xt[:], xT_d[i * 128:(i + 1) * 128, :])
                xTs.append(xt)
            # ---- y^T = Wo^T @ O^T + x^T ----
            yTs, yBs = [], []
            for i in range(NE):
                wo_t = w1pool.tile([128, NE, 128], BF16, tag="w1")
                nc.sync.dma_start(
                    wo_t[:], bass_rust.AP(wo_d, i * 128 * E,
                                          [[E, 128], [1, NE * 128]]))
                ps = ps_acc.tile([128, TPC], F32, tag="acc")
                for j in range(NE):
                    nc.tensor.matmul(ps[:], wo_t[:, j, :],
                                     oTs[j][:], start=(j == 0), stop=(j == NE - 1))
                yt = ypool.tile([128, TPC], F32R, tag=f"y{i}")
                nc.vector.tensor_add(yt[:], ps[:], xTs[i][:])
                yTs.append(yt)
                yb = ypool.tile([128, TPC], BF16, tag=f"yb{i}")
                nc.scalar.activation(yb[:], yt[:], AF.Copy)
                yBs.append(yb)

            # ---- r2 = 1/sqrt(mean(y^2)+eps) ----
            ssy_ps = ps_acc.tile([128, TPC], F32, tag="acc")
            for i in range(NE):
                sq = spool.tile([128, TPC], BF16, tag="sqy")
                nc.vector.tensor_mul(sq[:], yBs[i][:], yBs[i][:])
                nc.tensor.matmul(ssy_ps[0:1, :], ones128[:], sq[:],
                                 start=(i == 0), stop=(i == NE - 1))
            r2sq = spool.tile([1, TPC], F32, tag="r2sq")
            nc.scalar.activation(r2sq[:], ssy_ps[0:1, :], AF.Sqrt,
                                 bias=epsln_t[0:1, :], scale=1.0 / E)
            r2 = spool.tile([1, TPC], F32, tag="r2")
            nc.vector.reciprocal_approx_fast(r2[:], r2sq[:])
            # broadcast r2 over 128 partitions
            r2b_ps = ps_mm.tile([128, TPC], F32, tag="mm")
            o1x128 = cpool.tile([1, 128], F32)
            nc.sync.dma_start(o1x128[:], onesr_d[:])
            nc.tensor.matmul(r2b_ps[:], o1x128[:], r2[:], start=True, stop=True)
            r2b = cpool.tile([128, TPC], F32)
            nc.scalar.activation(r2b[:], r2b_ps[:], AF.Copy)

            # ---- h^T = relu(W1'^T y^T) ----
            hts = []
            for ht in range(NH):
                w1_t = w1pool.tile([128, NE, 128], BF16, tag="w1")
                nc.sync.dma_start(
                    w1_t[:], bass_rust.AP(w1_d, ht * 128 * E,
                                          [[E, 128], [1, NE * 128]]))
                ps = ps_mm.tile([128, TPC], F32, tag="mm")
                for j in range(NE):
                    nc.tensor.matmul(ps[:], w1_t[:, j, :], yBs[j][:],
                                     start=(j == 0), stop=(j == NE - 1))
                h_t = hpool.tile([128, TPC], BF16, tag=f"h{ht}")
                nc.scalar.activation(h_t[:], ps[:], AF.Relu)
                hts.append(h_t)

            # ---- z = (h^T' W2)^T * r2 + y ----
            for i in range(NE):
                w2_t = w2pool.tile([128, NH, 128], BF16, tag="w2")
                nc.sync.dma_start(
                    w2_t[:], bass_rust.AP(w2_d, i * 128 * HID,
                                          [[HID, 128], [1, NH * 128]]))
                ps = ps_acc.tile([128, TPC], F32, tag="acc")
                for ht in range(NH):
                    nc.tensor.matmul(ps[:], w2_t[:, ht, :], hts[ht][:],
                                     start=(ht == 0), stop=(ht == NH - 1))
                zt = spool.tile([128, TPC], F32, tag="zt")
                nc.vector.tensor_mul(zt[:], ps[:], r2b[:])
                outt = spool.tile([128, TPC], F32, tag="outt")
                nc.vector.tensor_add(outt[:], zt[:], yTs[i][:])
                nc.sync.dma_start(out_d[i * 128:(i + 1) * 128, :], outt[:])
    nc.compile()
    return nc


def prep_b_inputs(inputs, oT_all):
    x = np.asarray(inputs["x"], np.float32)
    ln2 = np.asarray(inputs["ln2_w"], np.float32)
    def tile_cols(w):
        # [K, M] -> [M//128, 128, K]: out[i, p, j*128+m] = w[j*128+p, i*128+m]
        K, M = w.shape
        return np.ascontiguousarray(
            w.reshape(K // 128, 128, M // 128, 128)
            .transpose(2, 1, 0, 3).reshape(M // 128, 128, K))
    Wo = tile_cols(np.asarray(inputs["Wo"], np.float32)).astype(NP_BF16)
    W1 = tile_cols(
        ln2[:, None] * np.asarray(inputs["W1"], np.float32)).astype(NP_BF16)
    W2 = tile_cols(np.asarray(inputs["W2"], np.float32)).astype(NP_BF16)
    xT = x.reshape(B * S, E).T
    in_maps = []
    for c in range(NCORE):
        ts = slice(c * TPC, (c + 1) * TPC)
        in_maps.append({
            "oTs": np.ascontiguousarray(oT_all[:, ts]).astype(NP_BF16),
            "onesc": np.ones((128, 1), NP_BF16),
            "onesr": np.ones((1, 128), np.float32),
            "xTs": np.ascontiguousarray(xT[:, ts]),
            "wo": Wo, "w1": W1, "w2": W2,
        })
    return in_maps


_CACHE = {}


def _get_compiled():
    if "a" not in _CACHE:
        _CACHE["a"] = build_launch_a()
    if "b" not in _CACHE:
        _CACHE["b"] = build_launch_b()
    return _CACHE["a"], _CACHE["b"]


def kernel(**inputs):
    from concourse import bass_utils
    inputs = {k: np.asarray(v) for k, v in inputs.items()}
    nca, ncb = _get_compiled()
    in_maps_a = prep_a_inputs(inputs)
    res_a = bass_utils.run_bass_kernel_spmd(
        nca, in_maps_a, core_ids=list(range(NCORE)))
    oT_all = np.concatenate([res_a.results[c]["oT"] for c in range(NCORE)],
                            axis=0)  # [E, B*S], rows = h*64+d
    in_maps_b = prep_b_inputs(inputs, oT_all)
    res_b = bass_utils.run_bass_kernel_spmd(
        ncb, in_maps_b, core_ids=list(range(NCORE)))
    outT = np.concatenate([res_b.results[c]["outT"] for c in range(NCORE)],
                          axis=1)  # [E, B*S]
    return np.ascontiguousarray(outT.T).reshape(B, S, E).astype(np.float32)



# revision 35
# speedup vs baseline: 1.8103x; 1.0230x over previous
# Trainium2 Bass kernels for nn_DecoderLayer (T5-style decoder layer).
# Two SPMD launches over 8 cores:
#   A: head-parallel attention (2 heads/core) -> per-head attn outputs O^T
#   B: token-parallel output-proj + FFN (512 tokens/core)
# Activations kept feature-major (features on partitions).
import sys
sys.path.insert(0, '/opt/trn_rl_repo')
import math
import numpy as np
import bass_rust
import concourse.bass as bass
import concourse.mybir as mybir
import concourse.tile as tile
from concourse import bacc

F32 = mybir.dt.float32
F32R = mybir.dt.float32r
BF16 = mybir.dt.bfloat16
FP8 = mybir.dt.float8e4
AF = mybir.ActivationFunctionType
DR = mybir.MatmulPerfMode.DoubleRow

import ml_dtypes
NP_BF16 = ml_dtypes.bfloat16
NP_FP8 = ml_dtypes.float8_e4m3fn

E = 1024
H = 16
D = 64
HID = 4096
B = 2
S = 2048
NB = 32
MAXD = 128
LN_EPS = 1e-6
KQ_EPS = 1e-6
NCORE = 8
HPC = H // NCORE          # heads per core (2)
TPC = B * S // NCORE      # tokens per core (512)
NQC = S // 512            # q-chunks per batch (4)
BOFF = 1024               # bias vec offset: index j = d + BOFF
MASKVAL = -20.0


def rel_bucket(d):
    d = np.asarray(d)
    max_exact = NB // 2
    safe = np.maximum(d, 1).astype(np.float64)
    large = max_exact + (
        np.log(safe / max_exact) / math.log(MAXD / max_exact) * (NB - max_exact)
    ).astype(np.int32)
    large = np.minimum(large, NB - 1)
    return np.where(d < max_exact, d, large)


def build_launch_a():
    nc = bacc.Bacc("TRN2", target_bir_lowering=False, debug=False)
    x8_d = nc.dram_tensor("x8", [E, B * S], FP8, kind="ExternalInput")
    wq_d = nc.dram_tensor("wq", [128, 8 * HPC * D], FP8, kind="ExternalInput").ap()
    wk_d = nc.dram_tensor("wk", [128, 8 * HPC * D], FP8, kind="ExternalInput").ap()
    wv_d = nc.dram_tensor("wv", [128, 8 * HPC * D], FP8, kind="ExternalInput").ap()
    expb_d = nc.dram_tensor("expb", [HPC * 5, 128, 512], BF16,
                            kind="ExternalInput")
    fconst_d = nc.dram_tensor("fconst", [128, HPC], F32, kind="ExternalInput").ap()
    scaleb_d = nc.dram_tensor("scaleb", [128, HPC], F32, kind="ExternalInput").ap()
    onesc_d = nc.dram_tensor("onesc", [128, 1], BF16, kind="ExternalInput").ap()
    hsum_d = nc.dram_tensor("hsum", [128, HPC], BF16, kind="ExternalInput").ap()
    hsel_d = nc.dram_tensor("hsel", [HPC, 128], BF16, kind="ExternalInput").ap()
    identc_d = nc.dram_tensor("identc", [128, 64], BF16, kind="ExternalInput").ap()
    oT_d = nc.dram_tensor("oT", [HPC * D, B * S], BF16, kind="ExternalOutput").ap()

    with tile.TileContext(nc) as tc:
        with nc.allow_low_precision(reason="bf16 kernel"), \
             tc.tile_pool(name="const", bufs=1) as cpool, \
             tc.tile_pool(name="xt", bufs=2) as xpool, \
             tc.tile_pool(name="w", bufs=1) as wpool, \
             tc.tile_pool(name="qkv", bufs=2) as qkvpool, \
             tc.tile_pool(name="vtok", bufs=2) as vpool, \
             tc.tile_pool(name="ptile", bufs=4) as ppool, \
             tc.tile_pool(name="onorm", bufs=3) as opool, \
             tc.tile_pool(name="small", bufs=3) as spool, \
             tc.tile_pool(name="r1p", bufs=2) as rpool, \
             tc.tile_pool(name="ps_pair", bufs=2, space="PSUM") as ps_pair, \
             tc.tile_pool(name="ps_sm", bufs=2, space="PSUM") as ps_sm, \
             tc.tile_pool(name="ps_acc", bufs=2, space="PSUM") as ps_acc:

            # ---- constants ----
            fconst_t = cpool.tile([128, HPC], F32)
            nc.sync.dma_start(fconst_t[:], fconst_d[:])
            scaleb_t = cpool.tile([128, HPC], F32)
            nc.sync.dma_start(scaleb_t[:], scaleb_d[:])
            ones128 = cpool.tile([128, 1], BF16)
            nc.sync.dma_start(ones128[:], onesc_d[:])
            hsum = cpool.tile([128, HPC], BF16)
            nc.sync.dma_start(hsum[:], hsum_d[:])
            hsel = cpool.tile([HPC, 128], BF16)
            nc.sync.dma_start(hsel[:], hsel_d[:])
            # r1 sqrt computes 64*sqrt(ms+eps) to fold away the x64 fp8
            # weight scaling on the v projection: scale=4096/E, bias=4096*eps
            epsln_t = cpool.tile([128, 1], F32)
            nc.vector.memset(epsln_t[:], LN_EPS * 4096.0)
            epskq_t = cpool.tile([128, 1], F32)
            nc.vector.memset(epskq_t[:], KQ_EPS)
            # stacked identities [128, 64] for per-head PE transpose
            ident = cpool.tile([128, 64], BF16)
            nc.sync.dma_start(ident[:], identc_d[:])
            # near-band exp(bias - fconst) tiles: [128, (h*5+di)*512 + f]
            expb = cpool.tile([128, HPC * 5 * 512], BF16)
            nc.sync.dma_start(
                expb[:],
                bass_rust.AP(expb_d, 0, [[512, 128], [65536, HPC * 5],
                                         [1, 512]]))

            # weights (ln1-folded, x64-scaled fp8): [128, e-tile, col]
            wq_t = wpool.tile([128, 8, HPC * D], FP8, tag="wq")
            wk_t = wpool.tile([128, 8, HPC * D], FP8, tag="wk")
            wv_t = wpool.tile([128, 8, HPC * D], FP8, tag="wv")
            for (w_t, w_d) in ((wq_t, wq_d), (wk_t, wk_d), (wv_t, wv_d)):
                nc.sync.dma_start(
                    w_t[:], w_d.rearrange("p (i m) -> p i m", i=8))

            for b in range(B):
                # fp8 x with all 8 e-tiles along free dim (DoubleRow pairs)
                x8 = xpool.tile([128, 8, S], FP8, tag="x8")
                nc.sync.dma_start(
                    x8[:], bass_rust.AP(x8_d, b * S,
                                        [[B * S, 128], [128 * B * S, 8],
                                         [1, S]]))

                # ---- r1 = 1/sqrt(mean(x^2)+eps) ----
                r1 = rpool.tile([1, S], F32, tag="r1")
                for ch in range(S // 512):
                    csl = slice(ch * 512, (ch + 1) * 512)
                    # per-etile squares, tree-added on DVE; single matmul
                    # reduces the 128 partitions.
                    ssq = spool.tile([128, 512], BF16, tag="ssq")
                    nc.vector.tensor_mul(ssq[:], x8[:, 0, csl],
                                         x8[:, 0, csl])
                    for i in range(1, 8):
                        sq = spool.tile([128, 512], BF16, tag="sqx")
                        nc.vector.tensor_mul(sq[:], x8[:, i, csl],
                                             x8[:, i, csl])
                        nc.vector.tensor_add(ssq[:], ssq[:], sq[:])
                    ssx_ps = ps_sm.tile([128, 512], F32, tag="sm")
                    nc.tensor.matmul(ssx_ps[0:1, :], ones128[:], ssq[:],
                                     start=True, stop=True)
                    r1sq = spool.tile([1, 512], F32, tag="r1sq")
                    nc.scalar.activation(r1sq[:], ssx_ps[0:1, :], AF.Sqrt,
                                         bias=epsln_t[0:1, :],
                                         scale=4096.0 / E)
                    nc.vector.reciprocal_approx_fast(r1[:, csl], r1sq[:])

                # ---- projections: raw qT/kT/vT packed [128, 2048] ----
                # i-outer loop reuses the stationary weight tile across the
                # 4 token chunks (2 psum pair-tiles of 2 chunks each).
                def project(w_t, name, copy_eng):
                    out = qkvpool.tile([128, S], BF16, tag=name)
                    pp0 = ps_pair.tile([128, 1024], F32, tag="pair")
                    pp1 = ps_pair.tile([128, 1024], F32, tag="pair")
                    pps = (pp0, pp1)
                    for jj in range(4):
                        for ch in range(4):
                            csl = slice(ch * 512, (ch + 1) * 512)
                            pp = pps[ch // 2]
                            off = (ch % 2) * 512
                            nc.tensor.matmul(
                                pp[:, off:off + 512],
                                w_t[:, 2 * jj:2 * jj + 2, :],
                                x8[:, 2 * jj:2 * jj + 2, csl],
                                start=(jj == 0), stop=(jj == 3),
                                perf_mode=DR)
                    for j in range(2):
                        osl = slice(j * 1024, (j + 1) * 1024)
                        if copy_eng == "scalar":
                            nc.scalar.activation(out[:, osl], pps[j][:],
                                                 AF.Copy)
                        else:
                            nc.vector.tensor_copy(out[:, osl], pps[j][:])
                    return out

                qT = project(wq_t, "qT", "vector")
                kT = project(wk_t, "kT", "vector")
                vT_s = project(wv_t, "vT", "vector")

                # ---- L2-normalize q and k in place ----
                # fp8 normalized q/k: [128, 2*S]; cols S..2S are zeros so a
                # [64, 2, n] slice pairs real data with a zero contraction
                # block for DoubleRow matmuls.
                qT8 = qkvpool.tile([128, 2 * S], FP8, tag="qT8")
                kT8 = qkvpool.tile([128, 2 * S], FP8, tag="kT8")
                nc.vector.memset(qT8[:, S:], 0.0)
                nc.vector.memset(kT8[:, S:], 0.0)

                def l2norm(raw, out8):
                    for ch in range(S // 512):
                        csl = slice(ch * 512, (ch + 1) * 512)
                        sq = spool.tile([128, 512], BF16, tag="sqn")
                        nc.vector.tensor_mul(sq[:], raw[:, csl], raw[:, csl])
                        ss_ps = ps_sm.tile([128, 512], F32, tag="sm")
                        nc.tensor.matmul(ss_ps[0:HPC, :], hsum[:], sq[:],
                                         start=True, stop=True)
                        rnsq = spool.tile([HPC, 512], F32, tag="rnsq")
                        nc.scalar.activation(rnsq[:], ss_ps[0:HPC, :], AF.Sqrt,
                                             bias=epskq_t[0:HPC, :])
                        rn = spool.tile([HPC, 512], F32, tag="rn")
                        nc.vector.reciprocal_approx_fast(rn[:], rnsq[:])
                        rnb = spool.tile([HPC, 512], BF16, tag="rnb")
                        nc.vector.tensor_copy(rnb[:], rn[:])
                        bc_ps = ps_sm.tile([128, 512], F32, tag="sm")
                        nc.tensor.matmul(bc_ps[:], hsel[:], rnb[:],
                                         start=True, stop=True)
                        nc.vector.tensor_mul(out8[:, csl], raw[:, csl],
                                             bc_ps[:])

                l2norm(qT, qT8)
                l2norm(kT, kT8)

                # ---- v scaled by r1 in place (feature-major) ----
                mvv = spool.tile([128, 1], F32, tag="mvv")
                mv4 = spool.tile([128, NQC], F32, tag="mv4")
                for ch in range(NQC):
                    csl = slice(ch * 512, (ch + 1) * 512)
                    bc = rpool.tile([128, 512], F32, tag="bc")
                    nc.gpsimd.partition_broadcast(bc[:], r1[:, csl],
                                                  channels=128)
                    nc.vector.tensor_mul(vT_s[:, csl], vT_s[:, csl], bc[:])
                    nc.vector.reduce_sum(mv4[:, ch:ch + 1], vT_s[:, csl],
                                         mybir.AxisListType.X)
                nc.vector.reduce_sum(mvv[:], mv4[:], mybir.AxisListType.X)

                v_augs = [[], []]
                for kt in range(S // 128):
                    for h in range(HPC):
                        va = vpool.tile([128, D + 1], BF16, tag=f"va{h}_{kt}")
                        nc.vector.tensor_copy(va[:, D:D + 1], ones128[:])
                        tr_ps = ps_sm.tile([128, 64], BF16, tag="sm")
                        nc.tensor.transpose(
                            tr_ps[0:128, 0:64],
                            vT_s[h * D:(h + 1) * D, kt * 128:(kt + 1) * 128],
                            ident[h * D:(h + 1) * D, :])
                        nc.vector.tensor_copy(va[:, 0:D], tr_ps[0:128, 0:64])
                        v_augs[h].append(va)

                # ---- attention ----
                # exp over paired k-tiles: p = exp(s*scale_h + fconst_h),
                # near-band tiles then multiplied by exp(bias - fconst).
                for h in range(HPC):
                    hd = slice(h * D, (h + 1) * D)
                    k8h = kT8[hd, :].rearrange("p (j c) -> p j c", j=2)
                    q8h = qT8[hd, :].rearrange("p (j c) -> p j c", j=2)
                    for qc in range(NQC):
                        Q0 = qc * 512
                        qsl = slice(Q0, Q0 + 512)
                        nkt = (Q0 + 512) // 128
                        o_ps = ps_acc.tile([128, 512], F32, tag="acc")
                        npair = nkt // 2

                        def emit_pv(pr, p_t):
                            for t2 in range(2):
                                kt = 2 * pr + t2
                                psl = slice(t2 * 512, (t2 + 1) * 512)
                                nc.tensor.matmul(
                                    o_ps[0:D + 1, :], v_augs[h][kt][:],
                                    p_t[:, psl],
                                    start=(kt == 0), stop=(kt == nkt - 1))

                        pending = []
                        for pr in range(npair):
                            pp = ps_pair.tile([128, 1024], F32, tag="pair")
                            for t2 in range(2):
                                K0 = (2 * pr + t2) * 128
                                nc.tensor.matmul(
                                    pp[:, t2 * 512:(t2 + 1) * 512],
                                    k8h[:, :, K0:K0 + 128],
                                    q8h[:, :, qsl],
                                    start=True, stop=True, perf_mode=DR)
                            p_t = ppool.tile([128, 1024], BF16, tag="p")
                            nc.scalar.activation(
                                p_t[:], pp[:], AF.Exp,
                                bias=fconst_t[:, h:h + 1],
                                scale=scaleb_t[:, h:h + 1])
                            for t2 in range(2):
                                kt = 2 * pr + t2
                                psl = slice(t2 * 512, (t2 + 1) * 512)
                                di = 1 + kt - 4 * qc
                                if 0 <= di <= 4:
                                    bsl = slice((h * 5 + di) * 512,
                                                (h * 5 + di + 1) * 512)
                                    nc.vector.tensor_mul(
                                        p_t[:, psl], p_t[:, psl],
                                        expb[:, bsl])
                            pending.append((pr, p_t))
                            # keep the PE one score-pair ahead of the PVs
                            if len(pending) >= 2:
                                emit_pv(*pending.pop(0))
                        for item in pending:
                            emit_pv(*item)
                        den = spool.tile([1, 512], F32, tag="den")
                        nc.vector.tensor_copy(den[:], o_ps[D:D + 1, :])
                        srec = spool.tile([1, 512], F32, tag="srec")
                        nc.vector.reciprocal_approx_fast(srec[:], den[:])
                        nb = rpool.tile([64, 512], F32, tag="nb")
                        nc.gpsimd.partition_broadcast(nb[:], srec[:],
                                                      channels=D)
                        o_n = opool.tile([64, 512], BF16, tag="on")
                        nc.vector.tensor_mul(o_n[:], o_ps[0:D, :], nb[:])
                        if qc == 0:
                            nc.vector.tensor_scalar_mul(
                                o_n[:, 0:1], mvv[h * D:(h + 1) * D, :],
                                1.0 / S)
                        nc.sync.dma_start(
                            oT_d[hd, b * S + Q0: b * S + Q0 + 512], o_n[:])
    nc.compile()
    return nc


def prep_a_inputs(inputs):
    x = np.asarray(inputs["x"], np.float32)
    ln1 = np.asarray(inputs["ln1_w"], np.float32)
    Wq = np.asarray(inputs["Wq"], np.float32)
    Wk = np.asarray(inputs["Wk"], np.float32)
    Wv = np.asarray(inputs["Wv"], np.float32)
    rb = np.asarray(inputs["rel_bias"], np.float32)
    scale = np.asarray(inputs["scale"], np.float32)
    xT = np.ascontiguousarray(x.reshape(B * S, E).T).astype(NP_BF16)
    d = np.arange(2048) - BOFF
    bucket = rel_bucket(np.maximum(d, 1))
    biasv_all = np.where(
        (d < 1)[None, :], np.float32(MASKVAL),
        scale[:, None] * rb[bucket, :].T.astype(np.float32)).astype(np.float32)
    fconst_all = (scale * rb[NB - 1, :]).astype(np.float32)
    # exp(bias - fconst) delta-tiles: [H, 5, 128, 512];
    # E[h,di,p,f] = exp(v_h[BOFF+128-128*di+f-p] - fconst_h)
    expv_all = np.exp(biasv_all - fconst_all[:, None]).astype(np.float32)
    di_ = np.arange(5)[:, None, None]
    p_ = np.arange(128)[None, :, None]
    f_ = np.arange(512)[None, None, :]
    idx = BOFF + 128 - 128 * di_ + f_ - p_
    expb_all = expv_all[:, idx].astype(NP_BF16)  # [H, 5, 128, 512]

    def tile_w(w):  # [1024, M] -> [128, 8*M]
        M = w.shape[1]
        return np.ascontiguousarray(
            w.reshape(8, 128, M).transpose(1, 0, 2).reshape(128, 8 * M))
    ident_np = np.concatenate([np.eye(D, dtype=np.float32)] * 2,
                              axis=0).astype(NP_BF16)
    in_maps = []
    for c in range(NCORE):
        hs = slice(c * HPC, (c + 1) * HPC)
        cs = slice(c * HPC * D, (c + 1) * HPC * D)
        hsum_np = np.zeros((128, HPC), np.float32)
        for h in range(HPC):
            hsum_np[h * D:(h + 1) * D, h] = 1.0
        hsel_np = np.zeros((HPC, 128), np.float32)
        for h in range(HPC):
            hsel_np[h, h * D:(h + 1) * D] = 1.0
        in_maps.append({
            "x8": xT.astype(NP_FP8),
            "hsel": hsel_np.astype(NP_BF16),
            "identc": ident_np,
            "onesc": np.ones((128, 1), NP_BF16),
            "hsum": hsum_np.astype(NP_BF16),
            "wq": tile_w(64.0 * ln1[:, None] * Wq[:, cs]).astype(NP_FP8),
            "wk": tile_w(64.0 * ln1[:, None] * Wk[:, cs]).astype(NP_FP8),
            "wv": tile_w(64.0 * ln1[:, None] * Wv[:, cs]).astype(NP_FP8),
            "expb": np.ascontiguousarray(
                expb_all[hs].reshape(HPC * 5, 128, 512)),
            "fconst": np.ascontiguousarray(
                np.broadcast_to(fconst_all[hs], (128, HPC))),
            "scaleb": np.ascontiguousarray(
                np.broadcast_to(scale[hs], (128, HPC))),
        })
    return in_maps


def build_launch_b():
    nc = bacc.Bacc("TRN2", target_bir_lowering=False, debug=False)
    oT_d = nc.dram_tensor("oTs", [E, TPC], BF16, kind="ExternalInput").ap()
    xT_d = nc.dram_tensor("xTs", [E, TPC], F32R, kind="ExternalInput").ap()
    wo_d = nc.dram_tensor("wo", [E // 128, 128, E], BF16, kind="ExternalInput")
    w1_d = nc.dram_tensor("w1", [HID // 128, 128, E], BF16, kind="ExternalInput")
    w2_d = nc.dram_tensor("w2", [E // 128, 128, HID], BF16, kind="ExternalInput")
    onesc_d = nc.dram_tensor("onesc", [128, 1], BF16, kind="ExternalInput").ap()
    onesr_d = nc.dram_tensor("onesr", [1, 128], F32, kind="ExternalInput").ap()
    out_d = nc.dram_tensor("outT", [E, TPC], F32, kind="ExternalOutput").ap()

    NE = E // 128    # 8 e-tiles
    NH = HID // 128  # 32 h-tiles

    with tile.TileContext(nc) as tc:
        with nc.allow_low_precision(reason="bf16 kernel"), \
             tc.tile_pool(name="const", bufs=1) as cpool, \
             tc.tile_pool(name="io", bufs=1) as iopool, \
             tc.tile_pool(name="y", bufs=1) as ypool, \
             tc.tile_pool(name="h", bufs=1) as hpool, \
             tc.tile_pool(name="w1s", bufs=3) as w1pool, \
             tc.tile_pool(name="w2s", bufs=3) as w2pool, \
             tc.tile_pool(name="small", bufs=2) as spool, \
             tc.tile_pool(name="ps_mm", bufs=4, space="PSUM") as ps_mm, \
             tc.tile_pool(name="ps_acc", bufs=2, space="PSUM") as ps_acc:

            ones128 = cpool.tile([128, 1], BF16)
            nc.sync.dma_start(ones128[:], onesc_d[:])
            epsln_t = cpool.tile([128, 1], F32)
            nc.vector.memset(epsln_t[:], LN_EPS)

            oTs, xTs = [], []
            for i in range(NE):
                ot = iopool.tile([128, TPC], BF16, tag=f"o{i}")
                nc.sync.dma_start(ot[:], oT_d[i * 128:(i + 1) * 128, :])
                oTs.append(ot)
                xt = iopool.tile([128, TPC], F32R, tag=f"xs{i}")
                nc.sync.dma_start(xt[:], xT_d[i * 128:(i + 1) * 128, :])
                xTs.append(xt)
            # ---- y^T = Wo^T @ O^T + x^T ----
            yTs, yBs = [], []
            for i in range(NE):
                wo_t = w1pool.tile([128, NE, 128], BF16, tag="w1")
                nc.sync.dma_start(
                    wo_t[:], bass_rust.AP(wo_d, i * 128 * E,
                                          [[E, 128], [1, NE * 128]]))
                ps = ps_acc.tile([128, TPC], F32, tag="acc")
                for j in range(NE):
                    nc.tensor.matmul(ps[:], wo_t[:, j, :],
                                     oTs[j][:], start=(j == 0), stop=(j == NE - 1))
                yt = ypool.tile([128, TPC], F32R, tag=f"y{i}")
                nc.vector.tensor_add(yt[:], ps[:], xTs[i][:])
                yTs.append(yt)
                yb = ypool.tile([128, TPC], BF16, tag=f"yb{i}")
                nc.scalar.activation(yb[:], yt[:], AF.Copy)
                yBs.append(yb)

            # ---- r2 = 1/sqrt(mean(y^2)+eps) ----
            ssy_ps = ps_acc.tile([128, TPC], F32, tag="acc")
            for i in range(NE):
                sq = spool.tile([128, TPC], BF16, tag="sqy")
                nc.vector.tensor_mul(sq[:], yBs[i][:], yBs[i][:])
                nc.tensor.matmul(ssy_ps[0:1, :], ones128[:], sq[:],
                                 start=(i == 0), stop=(i == NE - 1))
            r2sq = spool.tile([1, TPC], F32, tag="r2sq")
            nc.scalar.activation(r2sq[:], ssy_ps[0:1, :], AF.Sqrt,
                                 bias=epsln_t[0:1, :], scale=1.0 / E)
            r2 = spool.tile([1, TPC], F32, tag="r2")
            nc.vector.reciprocal_approx_fast(r2[:], r2sq[:])
            # broadcast r2 over 128 partitions
            r2b_ps = ps_mm.tile([128, TPC], F32, tag="mm")
            o1x128 = cpool.tile([1, 128], F32)
            nc.sync.dma_start(o1x128[:], onesr_d[:])
            nc.tensor.matmul(r2b_ps[:], o1x128[:], r2[:], start=True, stop=True)
            r2b = cpool.tile([128, TPC], F32)
            nc.scalar.activation(r2b[:], r2b_ps[:], AF.Copy)

            # ---- h^T = relu(W1'^T y^T) ----
            hts = []
            for ht in range(NH):
                w1_t = w1pool.tile([128, NE, 128], BF16, tag="w1")
                nc.sync.dma_start(
                    w1_t[:], bass_rust.AP(w1_d, ht * 128 * E,
                                          [[E, 128], [1, NE * 128]]))
                ps = ps_mm.tile([128, TPC], F32, tag="mm")
                for j in range(NE):
                    nc.tensor.matmul(ps[:], w1_t[:, j, :], yBs[j][:],
                                     start=(j == 0), stop=(j == NE - 1))
                h_t = hpool.tile([128, TPC], BF16, tag=f"h{ht}")
                nc.scalar.activation(h_t[:], ps[:], AF.Relu)
                hts.append(h_t)

            # ---- z = (h^T' W2)^T * r2 + y ----
            for i in range(NE):
                w2_t = w2pool.tile([128, NH, 128], BF16, tag="w2")
                nc.sync.dma_start(
                    w2_t[:], bass_rust.AP(w2_d, i * 128 * HID,
                                          [[HID, 128], [1, NH * 128]]))
                ps = ps_acc.tile([128, TPC], F32, tag="acc")
                for ht in range(NH):
                    nc.tensor.matmul(ps[:], w2_t[:, ht, :], hts[ht][:],
                                     start=(ht == 0), stop=(ht == NH - 1))
                zt = spool.tile([128, TPC], F32, tag="zt")
                nc.vector.tensor_mul(zt[:], ps[:], r2b[:])
                outt = spool.tile([128, TPC], F32, tag="outt")
                nc.vector.tensor_add(outt[:], zt[:], yTs[i][:])
                nc.sync.dma_start(out_d[i * 128:(i + 1) * 128, :], outt[:])
    nc.compile()
    return nc


def prep_b_inputs(inputs, oT_all):
    x = np.asarray(inputs["x"], np.float32)
    ln2 = np.asarray(inputs["ln2_w"], np.float32)
    def tile_cols(w):
        # [K, M] -> [M//128, 128, K]: out[i, p, j*128+m] = w[j*128+p, i*128+m]
        K, M = w.shape
        return np.ascontiguousarray(
            w.reshape(K // 128, 128, M // 128, 128)
            .transpose(2, 1, 0, 3).reshape(M // 128, 128, K))
    Wo = tile_cols(np.asarray(inputs["Wo"], np.float32)).astype(NP_BF16)
    W1 = tile_cols(
        ln2[:, None] * np.asarray(inputs["W1"], np.float32)).astype(NP_BF16)
    W2 = tile_cols(np.asarray(inputs["W2"], np.float32)).astype(NP_BF16)
    xT = x.reshape(B * S, E).T
    in_maps = []
    for c in range(NCORE):
        ts = slice(c * TPC, (c + 1) * TPC)
        in_maps.append({
            "oTs": np.ascontiguousarray(oT_all[:, ts]).astype(NP_BF16),
            "onesc": np.ones((128, 1), NP_BF16),
            "onesr": np.ones((1, 128), np.float32),
            "xTs": np.ascontiguousarray(xT[:, ts]),
            "wo": Wo, "w1": W1, "w2": W2,
        })
    return in_maps


_CACHE = {}


def _get_compiled():
    if "a" not in _CACHE:
        _CACHE["a"] = build_launch_a()
    if "b" not in _CACHE:
        _CACHE["b"] = build_launch_b()
    return _CACHE["a"], _CACHE["b"]


def kernel(**inputs):
    from concourse import bass_utils
    inputs = {k: np.asarray(v) for k, v in inputs.items()}
    nca, ncb = _get_compiled()
    in_maps_a = prep_a_inputs(inputs)
    res_a = bass_utils.run_bass_kernel_spmd(
        nca, in_maps_a, core_ids=list(range(NCORE)))
    oT_all = np.concatenate([res_a.results[c]["oT"] for c in range(NCORE)],
                            axis=0)  # [E, B*S], rows = h*64+d
    in_maps_b = prep_b_inputs(inputs, oT_all)
    res_b = bass_utils.run_bass_kernel_spmd(
        ncb, in_maps_b, core_ids=list(range(NCORE)))
    outT = np.concatenate([res_b.results[c]["outT"] for c in range(NCORE)],
                          axis=1)  # [E, B*S]
    return np.ascontiguousarray(outT.T).reshape(B, S, E).astype(np.float32)

